# revision 33
# baseline (speedup 1.0000x reference)
"""GRU-style GNN message-passing kernel for Trainium2 (8 NeuronCores, SPMD).

Reference computation (per node b, features 256, 8 neighbors):
    xr = x @ Wir.T + bir
    hr_n = hs_n @ Whr.T + bhr
    r_n = sigmoid(xr + hr_n)
    z = sigmoid(x @ Wiz.T + biz + h_sum @ Whz.T + bhz)
    s = sum_n r_n * hs_n
    n = tanh(x @ Win.T + bin + s @ Whn.T + bhn)
    out = (1 - z) * n + z * h_sum

Strategy: data-parallel over B=32768 across 8 cores (4096 rows each),
8 batch-chunks of 512 per core, feature-major on-chip layout
([256 features = 2 partition chunks of 128, batch free dim]).

Vs the bf16 baseline:
  - fp16 everywhere instead of bf16 (same engine throughput, 10-bit
    mantissa) which drops the base numeric error ~9x and frees error
    budget for:
  - fp8e4m3 DoubleRow matmuls for neighbor pairs 0..K_FP8_PAIRS-1 of
    the hr path: one 256-contraction DR matmul replaces two fp16
    passes (2x PE on those matmuls).  Whr is packed fp8 [f][k][m];
    hs additionally lands in SBUF as fp8 [pair,j,k,b] for those pairs.
  - hs on-chip layout [pair, j, k, b] (j-major) so the level-1 product
    tree add is one 1024-wide DVE op per pair.
  - no separate f32 h_sum copy: the combine reads the fp16 h_sum half
    of the packed x tile; output is stored fp16 and widened on host.
  - elementwise rebalance: combine sub+mul on Pool(gpsimd), final add
    on DVE.

The schedule keeps the chunk pipeline dense on PE (any PE idle gap
re-engages the HAM clock throttle): a 12-matmul front block (xr | z)
as the PE runway, neighbor pairs 0-1, the deferred n-gate of chunk
c-1, pairs 2-3; the product tree tail of chunk c runs at the front of
chunk c+1.
"""

import sys
import numpy as np
from contextlib import ExitStack

sys.path.insert(0, "/opt/trn_rl_repo")

import ml_dtypes
import concourse.bacc as bacc
import concourse.tile as tile
from concourse import mybir
from concourse.bass_utils import run_bass_kernel_spmd

F32 = mybir.dt.float32
F16 = mybir.dt.float16
FP8 = mybir.dt.float8e4
F16_NP = np.float16
F8_NP = ml_dtypes.float8_e4m3

N_NEIGH, B, IN, H = 8, 32768, 256, 256
M = 8                    # cores
BL = B // M              # rows per core (4096)
NCH = 8                  # batch chunks per core
CW = BL // NCH           # chunk width (512)
NPAIR = N_NEIGH // 2     # neighbor pairs (4)
K_FP8_PAIRS = 4          # pairs 0..K-1 use fp8 DoubleRow hr matmuls

_cached = None  # compiled program, reused across kernel() calls

SIG = mybir.ActivationFunctionType.Sigmoid
TANH = mybir.ActivationFunctionType.Tanh
DR = mybir.MatmulPerfMode.DoubleRow

# fp16 weight pack column offsets (need-ordered: xr gate, z gate,
# pair-3 whr, n).  Layout per weight: [k][f][m] (wcol below).
W_OFF = {"wir": 0, "wiz": 512, "whz": 1024, "whr": 1536, "win": 2176,
         "whn": 2688}
ID_OFF = 2048
WP_COLS = 3200
WP_SPLIT = 1536  # piece A: wir/wiz/whz; piece B: whr16/id/win/whn
# fp8 pack: whr for DR, [f][k][m]: col = f*256 + k*128 + m
WP8_COLS = 512


def _build():
    nc = bacc.Bacc("TRN2", target_bir_lowering=False, debug=False, num_devices=M)

    # xbL packs x (cols 0:1024) and h_sum (cols 1024:2048), fp16, k-major
    xbL = nc.dram_tensor("xbL", [NCH, 128, 2048], F16, kind="ExternalInput").ap()
    # hs fp16, per chunk [pair, j, k, b]
    hsL = nc.dram_tensor("hsL", [NCH, 128, 8192], F16, kind="ExternalInput").ap()
    # hs fp8 for DR pairs, per chunk [pair, j, k, b], pairs 0..K-1
    hs8L = nc.dram_tensor("hs8L", [NCH, 128, K_FP8_PAIRS * 2048], FP8,
                          kind="ExternalInput").ap()
    wpL = nc.dram_tensor("wpL", [128, WP_COLS], F16, kind="ExternalInput").ap()
    wp8L = nc.dram_tensor("wp8L", [128, WP8_COLS], FP8, kind="ExternalInput").ap()
    # bias pack: col f*3+j holds feature-chunk f of (b_r, b_z, b_n)[j]
    biasp = nc.dram_tensor("biasp", [128, 6], F32, kind="ExternalInput").ap()
    outL = nc.dram_tensor("outL", [NCH, 128, 1024], F16, kind="ExternalOutput").ap()

    with tile.TileContext(nc) as tc, ExitStack() as ctx:
        cpool = ctx.enter_context(tc.tile_pool(name="const", bufs=1))
        x_pool = ctx.enter_context(tc.tile_pool(name="x", bufs=3))
        hs_pool = ctx.enter_context(tc.tile_pool(name="hs", bufs=3))
        h8_pool = ctx.enter_context(tc.tile_pool(name="h8", bufs=3))
        xr_pool = ctx.enter_context(tc.tile_pool(name="xr", bufs=2))
        z_pool = ctx.enter_context(tc.tile_pool(name="z", bufs=2))
        rc_pool = ctx.enter_context(tc.tile_pool(name="rc", bufs=2))
        pd_pool = ctx.enter_context(tc.tile_pool(name="pd", bufs=2))
        sm_pool = ctx.enter_context(tc.tile_pool(name="sm", bufs=2))
        s_pool = ctx.enter_context(tc.tile_pool(name="s", bufs=2))
        n_pool = ctx.enter_context(tc.tile_pool(name="n", bufs=2))
        d_pool = ctx.enter_context(tc.tile_pool(name="d", bufs=2))
        o_pool = ctx.enter_context(tc.tile_pool(name="o", bufs=2))
        pp_pool = ctx.enter_context(tc.tile_pool(name="pp", bufs=2, space="PSUM"))

        # --- constants: weight packs + biases ---
        wp_t = cpool.tile([128, WP_COLS], F16, tag="wp", name="wp_t")
        nc.sync.dma_start(out=wp_t[:, 0:512], in_=wpL[:, 0:512])
        wp8_t = cpool.tile([128, WP8_COLS], FP8, tag="wp8", name="wp8_t")
        bias_t = cpool.tile([128, 6], F32, tag="biasp", name="bias_t")
        nc.sync.dma_start(out=bias_t[:, :], in_=biasp[:, :])

        # warm-up: the PE HAM clock-gate needs ~3.4us of sustained activity
        # to lift the 1.2GHz cold throttle.
        wu_t = cpool.tile([128, 128], F16, tag="wu", name="wu_t")
        nc.vector.memset(wu_t[:, :], 0)
        pwu = pp_pool.tile([128, 2048], F32, tag="pp", name="pwu")
        for i in range(44):
            nc.tensor.matmul(pwu[:, (i % 4) * 512:(i % 4) * 512 + 128],
                             wu_t[:, :], wu_t[:, :], start=True, stop=True)

        def wcol(w, k, f):  # stationary [128,128] for weight w, k-chunk, f-chunk
            off = W_OFF[w] + k * 256 + f * 128
            return wp_t[:, off:off + 128]

        def w8col(f):  # DR stationary [128, 2, 128] for whr fp8, f-chunk
            return wp8_t[:, f * 256:(f + 1) * 256].rearrange(
                "p (k m) -> p k m", k=2)

        id_t = wp_t[:, ID_OFF:ID_OFF + 128]

        state = {}  # chunk -> tiles needed by the deferred n-gate/combine

        def emit_ngate(c, fi, pn, st):
            o = pn[:, fi * 512:(fi + 1) * 512]
            nc.tensor.matmul(o, wcol("win", 0, fi), st["x"][:, 0:512],
                             start=True, stop=False)
            nc.tensor.matmul(o, wcol("win", 1, fi), st["x"][:, 512:1024],
                             start=False, stop=False)
            nc.tensor.matmul(o, wcol("whn", 0, fi), st["s"][:, 0:512],
                             start=False, stop=False)
            nc.tensor.matmul(o, wcol("whn", 1, fi), st["s"][:, 512:1024],
                             start=False, stop=True)

        def emit_mid(c):
            """Between pairs 1 and 2 of chunk c: the deferred n-gate of
            chunk c-1, then its combine (split Pool/DVE) + store."""
            st = state.pop(c - 1)
            pn = pp_pool.tile([128, 2048], F32, tag="pp", name=f"pn_{c - 1}")
            for fi in range(2):
                emit_ngate(c, fi, pn, st)
            nt = n_pool.tile([128, 1024], F16, tag="n", name=f"n_{c - 1}")
            for fi in range(2):
                nc.scalar.activation(nt[:, fi * 512:(fi + 1) * 512],
                                     pn[:, fi * 512:(fi + 1) * 512], TANH,
                                     bias=bias_t[:, fi * 3 + 2:fi * 3 + 3])
            # out = n + z * (h - n): all three ops on Pool, freeing DVE
            # for the xr drain it picked up from ACT
            dt_ = d_pool.tile([128, 1024], F16, tag="d", name=f"d_{c - 1}")
            with nc.allow_low_precision(reason="fp16 combine"):
                nc.gpsimd.tensor_sub(dt_[:, :], st["hf"][:, :], nt[:, :])
                nc.gpsimd.tensor_mul(dt_[:, :], st["z"][:, :], dt_[:, :])
                ot = o_pool.tile([128, 1024], F16, tag="o", name=f"o_{c - 1}")
                nc.gpsimd.tensor_add(ot[:, :], nt[:, :], dt_[:, :])
            nc.sync.dma_start(out=outL[c - 1], in_=ot[:, :])

        def emit_last_tail(cc, pra3, hsc, xrt, rct, pdt, smt):
            """Final chunk tail, fully f-split: for each feature half, the
            pair-3 sigmoid/product/fold chain completes that half of s,
            feeding the matching Whn contraction chunk immediately.  The
            s-independent Win matmuls issue first so PE stays busy while
            ACT/DVE work through the f0 chain."""
            st = state.pop(cc)
            base = 3 * 2048
            pn = pp_pool.tile([128, 2048], F32, tag="pp", name=f"pn_{cc}")
            for fi in range(2):
                o = pn[:, fi * 512:(fi + 1) * 512]
                nc.tensor.matmul(o, wcol("win", 0, fi), st["x"][:, 0:512],
                                 start=True, stop=False)
                nc.tensor.matmul(o, wcol("win", 1, fi), st["x"][:, 512:1024],
                                 start=False, stop=False)
            sct = s_pool.tile([128, 1024], F16, tag="s", name=f"s_{cc}")
            nt = n_pool.tile([128, 1024], F16, tag="n", name=f"n_{cc}")
            dt_ = d_pool.tile([128, 1024], F16, tag="d", name=f"d_{cc}")
            ot = o_pool.tile([128, 1024], F16, tag="o", name=f"o_{cc}")
            for fi in range(2):
                fb = fi * 512
                # pair-3 sigmoid + products, this feature half of both j
                for j in range(2):
                    blk = slice(base + j * 1024 + fb, base + j * 1024 + fb + 512)
                    nc.scalar.activation(rct[:, blk],
                                         pra3[:, j * 1024 + fb:
                                              j * 1024 + fb + 512], SIG)
                    with nc.allow_low_precision(reason="fp16 products"):
                        nc.vector.tensor_mul(pdt[:, blk], rct[:, blk],
                                             hsc[:, blk])
                with nc.allow_low_precision(reason="fp16 neighbor sums"):
                    nc.vector.tensor_add(smt[:, 3072 + fb:3072 + fb + 512],
                                         pdt[:, base + fb:base + fb + 512],
                                         pdt[:, base + 1024 + fb:
                                             base + 1024 + fb + 512])
                    nc.vector.tensor_add(sct[:, fb:fb + 512],
                                         smt[:, 1024 + fb:1024 + fb + 512],
                                         smt[:, 3072 + fb:3072 + fb + 512])
                # Whn contraction chunk fi feeds both output halves
                for fo in range(2):
                    nc.tensor.matmul(pn[:, fo * 512:(fo + 1) * 512],
                                     wcol("whn", fi, fo), sct[:, fb:fb + 512],
                                     start=False, stop=(fi == 1))
            for fi in range(2):
                s_ = slice(fi * 512, (fi + 1) * 512)
                nc.scalar.activation(nt[:, s_], pn[:, s_], TANH,
                                     bias=bias_t[:, fi * 3 + 2:fi * 3 + 3])
                with nc.allow_low_precision(reason="fp16 combine"):
                    nc.vector.tensor_sub(dt_[:, s_], st["hf"][:, s_], nt[:, s_])
                    nc.vector.tensor_mul(dt_[:, s_], st["z"][:, s_], dt_[:, s_])
                    nc.vector.tensor_add(ot[:, s_], nt[:, s_], dt_[:, s_])
                nc.sync.dma_start(out=outL[cc][:, s_], in_=ot[:, s_])

        def emit_pair(c, p, hsc, hs8c, xrt, rct, pdt, smt, mm_only=False):
            """Neighbor pair p of chunk c.  PSUM pra layout [j, f, b]."""
            base = p * 2048
            pra = pp_pool.tile([128, 2048], F32, tag="pp", name=f"pr{p}_{c}")
            if p < K_FP8_PAIRS:
                for j in range(2):
                    mv = hs8c[:, base + j * 1024:base + (j + 1) * 1024]
                    mv3 = mv.rearrange("q (k b) -> q k b", k=2)
                    for fi in range(2):
                        nc.tensor.matmul(
                            pra[:, j * 1024 + fi * 512:j * 1024 + (fi + 1) * 512],
                            w8col(fi), mv3, start=True, stop=False,
                            perf_mode=DR)
            else:
                for j in range(2):
                    jb = base + j * 1024
                    for fi in range(2):
                        o = pra[:, j * 1024 + fi * 512:j * 1024 + (fi + 1) * 512]
                        nc.tensor.matmul(o, wcol("whr", 0, fi),
                                         hsc[:, jb:jb + 512],
                                         start=True, stop=False)
                        nc.tensor.matmul(o, wcol("whr", 1, fi),
                                         hsc[:, jb + 512:jb + 1024],
                                         start=False, stop=False)
            # xr identity adds (512-wide: matmul out must fit a PSUM bank)
            for j in range(2):
                for fi in range(2):
                    nc.tensor.matmul(
                        pra[:, j * 1024 + fi * 512:j * 1024 + (fi + 1) * 512],
                        id_t, xrt[:, fi * 512:(fi + 1) * 512],
                        start=False, stop=True)
            if mm_only:
                return pra
            # r for pair p, both neighbors in one activation
            nc.scalar.activation(rct[:, base:base + 2048], pra[:, :], SIG)
            blk = slice(base, base + 2048)
            with nc.allow_low_precision(reason="fp16 products"):
                nc.vector.tensor_mul(pdt[:, blk], rct[:, blk], hsc[:, blk])
            # tree level 1: j0 + j1 -> smt cols [p*1024, +1024)
            with nc.allow_low_precision(reason="fp16 neighbor sums"):
                nc.vector.tensor_add(smt[:, p * 1024:(p + 1) * 1024],
                                     pdt[:, base:base + 1024],
                                     pdt[:, base + 1024:base + 2048])
            return pra

        pend = {}  # chunk -> smt awaiting tree levels 2+3

        def emit_l23(cc):
            smt = pend.pop(cc)
            sct = s_pool.tile([128, 1024], F16, tag="s", name=f"s_{cc}")
            with nc.allow_low_precision(reason="fp16 neighbor sums"):
                nc.vector.tensor_add(smt[:, 0:1024], smt[:, 0:1024],
                                     smt[:, 1024:2048])
                nc.vector.tensor_add(smt[:, 2048:3072], smt[:, 2048:3072],
                                     smt[:, 3072:4096])
                nc.vector.tensor_add(sct[:, :], smt[:, 0:1024],
                                     smt[:, 2048:3072])
            state[cc]["s"] = sct

        for c in range(NCH):
            xbt = x_pool.tile([128, 2048], F16, tag="x", name=f"x_{c}")
            hsc = hs_pool.tile([128, 8192], F16, tag="hs", name=f"hs_{c}")
            hs8c = h8_pool.tile([128, K_FP8_PAIRS * 2048], FP8, tag="h8",
                                name=f"h8_{c}")
            if c == 0:
                nc.sync.dma_start(out=xbt[:, 0:1024], in_=xbL[c][:, 0:1024])
                nc.sync.dma_start(out=wp_t[:, 512:WP_SPLIT],
                                  in_=wpL[:, 512:WP_SPLIT])
                nc.sync.dma_start(out=xbt[:, 1024:2048],
                                  in_=xbL[c][:, 1024:2048])
                nc.sync.dma_start(out=wp8_t[:, :], in_=wp8L[:, :])
                nc.sync.dma_start(out=hs8c[:, 0:2048], in_=hs8L[c][:, 0:2048])
                nc.sync.dma_start(out=wp_t[:, WP_SPLIT:WP_COLS],
                                  in_=wpL[:, WP_SPLIT:WP_COLS])
                nc.sync.dma_start(out=hs8c[:, 2048:],
                                  in_=hs8L[c][:, 2048:])
                nc.sync.dma_start(out=hsc[:, :], in_=hsL[c])
            else:
                nc.sync.dma_start(out=xbt[:, :], in_=xbL[c])
                nc.sync.dma_start(out=hs8c[:, :], in_=hs8L[c])
                nc.sync.dma_start(out=hsc[:, :], in_=hsL[c])

            pg = pp_pool.tile([128, 2048], F32, tag="pp", name=f"pg_{c}")
            for fi in range(2):
                o = pg[:, fi * 512:(fi + 1) * 512]
                nc.tensor.matmul(o, wcol("wir", 0, fi), xbt[:, 0:512],
                                 start=True, stop=False)
                nc.tensor.matmul(o, wcol("wir", 1, fi), xbt[:, 512:1024],
                                 start=False, stop=True)
            # xr drain on DVE (first in its chunk queue, ahead of the l23
            # adds) -- frees 1.4us of the chunk's ACT critical chain, which
            # paces the whole schedule
            xrt = xr_pool.tile([128, 1024], F16, tag="xr", name=f"xr_{c}")
            with nc.allow_low_precision(reason="fp16 xr"):
                for fi in range(2):
                    nc.vector.tensor_scalar_add(
                        xrt[:, fi * 512:(fi + 1) * 512],
                        pg[:, fi * 512:(fi + 1) * 512],
                        bias_t[:, fi * 3:fi * 3 + 1])
            for fi in range(2):
                o = pg[:, 1024 + fi * 512:1024 + (fi + 1) * 512]
                nc.tensor.matmul(o, wcol("wiz", 0, fi), xbt[:, 0:512],
                                 start=True, stop=False)
                nc.tensor.matmul(o, wcol("wiz", 1, fi), xbt[:, 512:1024],
                                 start=False, stop=False)
                nc.tensor.matmul(o, wcol("whz", 0, fi), xbt[:, 1024:1536],
                                 start=False, stop=False)
                nc.tensor.matmul(o, wcol("whz", 1, fi), xbt[:, 1536:2048],
                                 start=False, stop=True)
            # tree tail of the previous chunk, after xr in the DVE queue
            if c > 0:
                emit_l23(c - 1)
            zt = z_pool.tile([128, 1024], F16, tag="z", name=f"z_{c}")
            for fi in range(2):
                nc.scalar.activation(zt[:, fi * 512:(fi + 1) * 512],
                                     pg[:, 1024 + fi * 512:1024 + (fi + 1) * 512],
                                     SIG, bias=bias_t[:, fi * 3 + 1:fi * 3 + 2])

            rct = rc_pool.tile([128, 4 * 2048], F16, tag="rc", name=f"rc_{c}")
            pdt = pd_pool.tile([128, 4 * 2048], F16, tag="pd", name=f"pd_{c}")
            smt = sm_pool.tile([128, 4 * 1024], F16, tag="sm", name=f"sm_{c}")
            state[c] = {"x": xbt, "hf": xbt[:, 1024:2048], "z": zt}
            emit_pair(c, 0, hsc, hs8c, xrt, rct, pdt, smt)
            emit_pair(c, 1, hsc, hs8c, xrt, rct, pdt, smt)
            if c == NCH - 1:
                with nc.allow_low_precision(reason="fp16 neighbor sums"):
                    nc.vector.tensor_add(smt[:, 0:1024], smt[:, 0:1024],
                                         smt[:, 1024:2048])
            if c > 0:
                emit_mid(c)
            emit_pair(c, 2, hsc, hs8c, xrt, rct, pdt, smt)
            if c == NCH - 1:
                with nc.allow_low_precision(reason="fp16 neighbor sums"):
                    nc.vector.tensor_add(smt[:, 1024:2048], smt[:, 0:1024],
                                         smt[:, 2048:3072])
            if c == NCH - 1:
                pra3 = emit_pair(c, 3, hsc, hs8c, xrt, rct, pdt, smt,
                                 mm_only=True)
                emit_last_tail(c, pra3, hsc, xrt, rct, pdt, smt)
            else:
                emit_pair(c, 3, hsc, hs8c, xrt, rct, pdt, smt)
                pend[c] = smt

    nc.compile()
    return nc


def _prep_inputs(x, h_sum, hs, Wir, bir, Whr, bhr, Wiz, biz, Whz, bhz,
                 Win, bin_, Whn, bhn):
    """Shard + pre-chunk to per-core, per-chunk feature-major HBM layouts."""
    f32 = np.float32
    x = np.asarray(x, f32)
    h = np.asarray(h_sum, f32)
    hs = np.asarray(hs, f32)

    wpack = np.zeros((128, WP_COLS), f32)
    for w, W in (("wir", Wir), ("whr", Whr), ("wiz", Wiz), ("whz", Whz),
                 ("win", Win), ("whn", Whn)):
        WT = np.asarray(W, f32).T  # [in, out]
        for k in range(2):
            wpack[:, W_OFF[w] + k * 256:W_OFF[w] + (k + 1) * 256] = \
                WT[k * 128:(k + 1) * 128, :]
    wpack[:, ID_OFF:ID_OFF + 128] = np.eye(128, dtype=f32)
    wpack_f16 = np.ascontiguousarray(wpack.astype(F16_NP))

    WhrT = np.asarray(Whr, f32).T
    wp8 = np.zeros((128, WP8_COLS), f32)
    for f in range(2):
        for k in range(2):
            wp8[:, f * 256 + k * 128:f * 256 + (k + 1) * 128] = \
                WhrT[k * 128:(k + 1) * 128, f * 128:(f + 1) * 128]
    wp8_f8 = np.ascontiguousarray(wp8.astype(F8_NP))

    b_r = np.asarray(bir, f32) + np.asarray(bhr, f32)
    b_z = np.asarray(biz, f32) + np.asarray(bhz, f32)
    b_n = np.asarray(bin_, f32) + np.asarray(bhn, f32)
    biasp = np.empty((128, 6), f32)
    for f in range(2):
        biasp[:, f * 3 + 0] = b_r[f * 128:(f + 1) * 128]
        biasp[:, f * 3 + 1] = b_z[f * 128:(f + 1) * 128]
        biasp[:, f * 3 + 2] = b_n[f * 128:(f + 1) * 128]

    in_maps = []
    for c in range(M):
        sl = slice(c * BL, (c + 1) * BL)
        xc = x[sl].reshape(NCH, CW, 2, 128).transpose(0, 3, 2, 1)
        hc = h[sl].reshape(NCH, CW, 2, 128).transpose(0, 3, 2, 1)
        xb = np.concatenate([xc.astype(F16_NP).reshape(NCH, 128, 1024),
                             hc.astype(F16_NP).reshape(NCH, 128, 1024)], axis=2)
        # hs: [8, BL, 256] -> [pr, j, ch, b, k, p] -> [ch, p, pr, j, k, b]
        hsc = hs[:, sl, :].reshape(NPAIR, 2, NCH, CW, 2, 128)
        hs_t = hsc.transpose(2, 5, 0, 1, 4, 3)  # [ch, p, pr, j, k, b]
        m = {
            "xbL": np.ascontiguousarray(xb),
            "hsL": np.ascontiguousarray(
                hs_t.astype(F16_NP).reshape(NCH, 128, 8192)),
            "hs8L": np.ascontiguousarray(
                hs_t[:, :, :K_FP8_PAIRS].astype(F8_NP).reshape(
                    NCH, 128, K_FP8_PAIRS * 2048)),
            "wpL": wpack_f16,
            "wp8L": wp8_f8,
            "biasp": biasp,
        }
        in_maps.append(m)
    return in_maps


def _run(inputs, trace=False, **trace_kwargs):
    global _cached
    if _cached is None:
        _cached = _build()
    nc = _cached
    in_maps = _prep_inputs(**inputs)
    res = run_bass_kernel_spmd(nc, in_maps, list(range(M)), trace=trace,
                               **trace_kwargs)
    out = np.empty((B, H), np.float32)
    for c in range(M):
        # outL [ch, p, (f b)] -> [ch, b, f, p] -> [BL, 256]
        o = np.asarray(res.results[c]["outL"], np.float32).reshape(
            NCH, 128, 2, CW)
        out[c * BL:(c + 1) * BL, :] = o.transpose(0, 3, 2, 1).reshape(BL, 256)
    return out, res


def kernel(**inputs):
    return _run(inputs)[0]


# revision 35
# speedup vs baseline: 1.3691x; 1.3691x over previous
"""GRU-style GNN message-passing kernel for Trainium2 (8 NeuronCores, SPMD).

Reference computation (per node b, features 256, 8 neighbors):
    xr = x @ Wir.T + bir
    hr_n = hs_n @ Whr.T + bhr
    r_n = sigmoid(xr + hr_n)
    z = sigmoid(x @ Wiz.T + biz + h_sum @ Whz.T + bhz)
    s = sum_n r_n * hs_n
    n = tanh(x @ Win.T + bin + s @ Whn.T + bhn)
    out = (1 - z) * n + z * h_sum

Strategy: data-parallel over B=32768 across 8 cores (4096 rows each),
8 batch-chunks of 512 per core, feature-major on-chip layout
([256 features = 2 partition chunks of 128, batch free dim]).

Vs the bf16 baseline:
  - fp16 everywhere instead of bf16 (same engine throughput, 10-bit
    mantissa) which drops the base numeric error ~9x and frees error
    budget for:
  - fp8e4m3 DoubleRow matmuls for neighbor pairs 0..K_FP8_PAIRS-1 of
    the hr path: one 256-contraction DR matmul replaces two fp16
    passes (2x PE on those matmuls).  Whr is packed fp8 [f][k][m];
    hs additionally lands in SBUF as fp8 [pair,j,k,b] for those pairs.
  - hs on-chip layout [pair, j, k, b] (j-major) so the level-1 product
    tree add is one 1024-wide DVE op per pair.
  - no separate f32 h_sum copy: the combine reads the fp16 h_sum half
    of the packed x tile; output is stored fp16 and widened on host.
  - elementwise rebalance: combine sub+mul on Pool(gpsimd), final add
    on DVE.

The schedule keeps the chunk pipeline dense on PE (any PE idle gap
re-engages the HAM clock throttle): a 12-matmul front block (xr | z)
as the PE runway, neighbor pairs 0-1, the deferred n-gate of chunk
c-1, pairs 2-3; the product tree tail of chunk c runs at the front of
chunk c+1.
"""

import sys
import numpy as np
from contextlib import ExitStack

sys.path.insert(0, "/opt/trn_rl_repo")

import ml_dtypes
import concourse.bacc as bacc
import concourse.tile as tile
from concourse import mybir
from concourse.bass_utils import run_bass_kernel_spmd

F32 = mybir.dt.float32
F16 = mybir.dt.float16
FP8 = mybir.dt.float8e4
F16_NP = np.float16
F8_NP = ml_dtypes.float8_e4m3

N_NEIGH, B, IN, H = 8, 32768, 256, 256
M = 8                    # cores
BL = B // M              # rows per core (4096)
NCH = 8                  # batch chunks per core
CW = BL // NCH           # chunk width (512)
NPAIR = N_NEIGH // 2     # neighbor pairs (4)
K_FP8_PAIRS = 4          # pairs 0..K-1 use fp8 DoubleRow hr matmuls

_cached = None  # compiled program, reused across kernel() calls

SIG = mybir.ActivationFunctionType.Sigmoid
TANH = mybir.ActivationFunctionType.Tanh
DR = mybir.MatmulPerfMode.DoubleRow

# fp16 weight pack column offsets (need-ordered: xr gate, z gate,
# pair-3 whr, n).  Layout per weight: [k][f][m] (wcol below).
W_OFF = {"wir": 0, "wiz": 512, "whz": 1024, "whr": 1536, "win": 2176,
         "whn": 2688}
ID_OFF = 2048
WP_COLS = 3200
WP_SPLIT = 1536  # piece A: wir/wiz/whz; piece B: whr16/id/win/whn
# fp8 pack: whr for DR, [f][k][m]: col = f*256 + k*128 + m
WP8_COLS = 512


def _build():
    nc = bacc.Bacc("TRN2", target_bir_lowering=False, debug=False, num_devices=M)

    # xbL packs x (cols 0:1024) and h_sum (cols 1024:2048), fp16, k-major
    xbL = nc.dram_tensor("xbL", [NCH, 128, 2048], F16, kind="ExternalInput").ap()
    # hs fp16, per chunk [pair, j, k, b]
    hsL = nc.dram_tensor("hsL", [NCH, 128, 8192], F16, kind="ExternalInput").ap()
    # hs fp8 for DR pairs, per chunk [pair, j, k, b], pairs 0..K-1
    hs8L = nc.dram_tensor("hs8L", [NCH, 128, K_FP8_PAIRS * 2048], FP8,
                          kind="ExternalInput").ap()
    wpL = nc.dram_tensor("wpL", [128, WP_COLS], F16, kind="ExternalInput").ap()
    wp8L = nc.dram_tensor("wp8L", [128, WP8_COLS], FP8, kind="ExternalInput").ap()
    # bias pack: col f*3+j holds feature-chunk f of (b_r, b_z, b_n)[j]
    biasp = nc.dram_tensor("biasp", [128, 6], F32, kind="ExternalInput").ap()
    outL = nc.dram_tensor("outL", [NCH, 128, 1024], F16, kind="ExternalOutput").ap()

    with tile.TileContext(nc) as tc, ExitStack() as ctx:
        cpool = ctx.enter_context(tc.tile_pool(name="const", bufs=1))
        x_pool = ctx.enter_context(tc.tile_pool(name="x", bufs=3))
        hs_pool = ctx.enter_context(tc.tile_pool(name="hs", bufs=3))
        h8_pool = ctx.enter_context(tc.tile_pool(name="h8", bufs=3))
        xr_pool = ctx.enter_context(tc.tile_pool(name="xr", bufs=2))
        z_pool = ctx.enter_context(tc.tile_pool(name="z", bufs=2))
        rc_pool = ctx.enter_context(tc.tile_pool(name="rc", bufs=2))
        pd_pool = ctx.enter_context(tc.tile_pool(name="pd", bufs=2))
        sm_pool = ctx.enter_context(tc.tile_pool(name="sm", bufs=2))
        s_pool = ctx.enter_context(tc.tile_pool(name="s", bufs=2))
        n_pool = ctx.enter_context(tc.tile_pool(name="n", bufs=2))
        d_pool = ctx.enter_context(tc.tile_pool(name="d", bufs=2))
        o_pool = ctx.enter_context(tc.tile_pool(name="o", bufs=2))
        pp_pool = ctx.enter_context(tc.tile_pool(name="pp", bufs=2, space="PSUM"))

        # --- constants: weight packs + biases ---
        wp_t = cpool.tile([128, WP_COLS], F16, tag="wp", name="wp_t")
        nc.sync.dma_start(out=wp_t[:, 0:512], in_=wpL[:, 0:512])
        wp8_t = cpool.tile([128, WP8_COLS], FP8, tag="wp8", name="wp8_t")
        bias_t = cpool.tile([128, 6], F32, tag="biasp", name="bias_t")
        nc.sync.dma_start(out=bias_t[:, :], in_=biasp[:, :])

        # warm-up: the PE HAM clock-gate needs ~3.4us of sustained activity
        # to lift the 1.2GHz cold throttle.
        wu_t = cpool.tile([128, 128], F16, tag="wu", name="wu_t")
        nc.vector.memset(wu_t[:, :], 0)
        pwu = pp_pool.tile([128, 2048], F32, tag="pp", name="pwu")
        for i in range(44):
            nc.tensor.matmul(pwu[:, (i % 4) * 512:(i % 4) * 512 + 128],
                             wu_t[:, :], wu_t[:, :], start=True, stop=True)

        def wcol(w, k, f):  # stationary [128,128] for weight w, k-chunk, f-chunk
            off = W_OFF[w] + k * 256 + f * 128
            return wp_t[:, off:off + 128]

        def w8col(f):  # DR stationary [128, 2, 128] for whr fp8, f-chunk
            return wp8_t[:, f * 256:(f + 1) * 256].rearrange(
                "p (k m) -> p k m", k=2)

        id_t = wp_t[:, ID_OFF:ID_OFF + 128]

        state = {}  # chunk -> tiles needed by the deferred n-gate/combine

        def emit_ngate(c, fi, pn, st):
            o = pn[:, fi * 512:(fi + 1) * 512]
            nc.tensor.matmul(o, wcol("win", 0, fi), st["x"][:, 0:512],
                             start=True, stop=False)
            nc.tensor.matmul(o, wcol("win", 1, fi), st["x"][:, 512:1024],
                             start=False, stop=False)
            nc.tensor.matmul(o, wcol("whn", 0, fi), st["s"][:, 0:512],
                             start=False, stop=False)
            nc.tensor.matmul(o, wcol("whn", 1, fi), st["s"][:, 512:1024],
                             start=False, stop=True)

        def emit_mid(c):
            """Between pairs 1 and 2 of chunk c: the deferred n-gate of
            chunk c-1, then its combine (split Pool/DVE) + store."""
            st = state.pop(c - 1)
            pn = pp_pool.tile([128, 2048], F32, tag="pp", name=f"pn_{c - 1}")
            for fi in range(2):
                emit_ngate(c, fi, pn, st)
            nt = n_pool.tile([128, 1024], F16, tag="n", name=f"n_{c - 1}")
            for fi in range(2):
                nc.scalar.activation(nt[:, fi * 512:(fi + 1) * 512],
                                     pn[:, fi * 512:(fi + 1) * 512], TANH,
                                     bias=bias_t[:, fi * 3 + 2:fi * 3 + 3])
            # out = n + z * (h - n): all three ops on Pool, freeing ~0.7us
            # of DVE per chunk (DVE runs ~92% busy at the chunk cadence)
            dt_ = d_pool.tile([128, 1024], F16, tag="d", name=f"d_{c - 1}")
            with nc.allow_low_precision(reason="fp16 combine"):
                nc.gpsimd.tensor_sub(dt_[:, :], st["hf"][:, :], nt[:, :])
                nc.gpsimd.tensor_mul(dt_[:, :], st["z"][:, :], dt_[:, :])
                ot = o_pool.tile([128, 1024], F16, tag="o", name=f"o_{c - 1}")
                nc.gpsimd.tensor_add(ot[:, :], nt[:, :], dt_[:, :])
            nc.sync.dma_start(out=outL[c - 1], in_=ot[:, :])

        def emit_last_tail(cc, pra3, hsc, xrt, rct, pdt, smt):
            """Final chunk tail, fully f-split: for each feature half, the
            pair-3 sigmoid/product/fold chain completes that half of s,
            feeding the matching Whn contraction chunk immediately.  The
            s-independent Win matmuls issue first so PE stays busy while
            ACT/DVE work through the f0 chain."""
            st = state.pop(cc)
            base = 3 * 2048
            pn = pp_pool.tile([128, 2048], F32, tag="pp", name=f"pn_{cc}")
            for fi in range(2):
                o = pn[:, fi * 512:(fi + 1) * 512]
                nc.tensor.matmul(o, wcol("win", 0, fi), st["x"][:, 0:512],
                                 start=True, stop=False)
                nc.tensor.matmul(o, wcol("win", 1, fi), st["x"][:, 512:1024],
                                 start=False, stop=False)
            sct = s_pool.tile([128, 1024], F16, tag="s", name=f"s_{cc}")
            nt = n_pool.tile([128, 1024], F16, tag="n", name=f"n_{cc}")
            dt_ = d_pool.tile([128, 1024], F16, tag="d", name=f"d_{cc}")
            ot = o_pool.tile([128, 1024], F16, tag="o", name=f"o_{cc}")
            for fi in range(2):
                fb = fi * 512
                # pair-3 sigmoid + products, this feature half of both j
                for j in range(2):
                    blk = slice(base + j * 1024 + fb, base + j * 1024 + fb + 512)
                    nc.scalar.activation(rct[:, blk],
                                         pra3[:, j * 1024 + fb:
                                              j * 1024 + fb + 512], SIG)
                    with nc.allow_low_precision(reason="fp16 products"):
                        nc.vector.tensor_mul(pdt[:, blk], rct[:, blk],
                                             hsc[:, blk])
                with nc.allow_low_precision(reason="fp16 neighbor sums"):
                    nc.vector.tensor_add(smt[:, 3072 + fb:3072 + fb + 512],
                                         pdt[:, base + fb:base + fb + 512],
                                         pdt[:, base + 1024 + fb:
                                             base + 1024 + fb + 512])
                    nc.vector.tensor_add(sct[:, fb:fb + 512],
                                         smt[:, 1024 + fb:1024 + fb + 512],
                                         smt[:, 3072 + fb:3072 + fb + 512])
                # Whn contraction chunk fi feeds both output halves
                for fo in range(2):
                    nc.tensor.matmul(pn[:, fo * 512:(fo + 1) * 512],
                                     wcol("whn", fi, fo), sct[:, fb:fb + 512],
                                     start=False, stop=(fi == 1))
            for fi in range(2):
                s_ = slice(fi * 512, (fi + 1) * 512)
                nc.scalar.activation(nt[:, s_], pn[:, s_], TANH,
                                     bias=bias_t[:, fi * 3 + 2:fi * 3 + 3])
                with nc.allow_low_precision(reason="fp16 combine"):
                    nc.vector.tensor_sub(dt_[:, s_], st["hf"][:, s_], nt[:, s_])
                    nc.vector.tensor_mul(dt_[:, s_], st["z"][:, s_], dt_[:, s_])
                    nc.vector.tensor_add(ot[:, s_], nt[:, s_], dt_[:, s_])
                nc.sync.dma_start(out=outL[cc][:, s_], in_=ot[:, s_])

        def emit_pair(c, p, hsc, hs8c, xrt, rct, pdt, smt, mm_only=False):
            """Neighbor pair p of chunk c.  PSUM pra layout [j, f, b]."""
            base = p * 2048
            pra = pp_pool.tile([128, 2048], F32, tag="pp", name=f"pr{p}_{c}")
            if p < K_FP8_PAIRS:
                for j in range(2):
                    mv = hs8c[:, base + j * 1024:base + (j + 1) * 1024]
                    mv3 = mv.rearrange("q (k b) -> q k b", k=2)
                    for fi in range(2):
                        nc.tensor.matmul(
                            pra[:, j * 1024 + fi * 512:j * 1024 + (fi + 1) * 512],
                            w8col(fi), mv3, start=True, stop=False,
                            perf_mode=DR)
            else:
                for j in range(2):
                    jb = base + j * 1024
                    for fi in range(2):
                        o = pra[:, j * 1024 + fi * 512:j * 1024 + (fi + 1) * 512]
                        nc.tensor.matmul(o, wcol("whr", 0, fi),
                                         hsc[:, jb:jb + 512],
                                         start=True, stop=False)
                        nc.tensor.matmul(o, wcol("whr", 1, fi),
                                         hsc[:, jb + 512:jb + 1024],
                                         start=False, stop=False)
            # xr identity adds (512-wide: matmul out must fit a PSUM bank)
            for j in range(2):
                for fi in range(2):
                    nc.tensor.matmul(
                        pra[:, j * 1024 + fi * 512:j * 1024 + (fi + 1) * 512],
                        id_t, xrt[:, fi * 512:(fi + 1) * 512],
                        start=False, stop=True)
            if mm_only:
                return pra
            # r for pair p, both neighbors in one activation
            nc.scalar.activation(rct[:, base:base + 2048], pra[:, :], SIG)
            blk = slice(base, base + 2048)
            with nc.allow_low_precision(reason="fp16 products"):
                nc.vector.tensor_mul(pdt[:, blk], rct[:, blk], hsc[:, blk])
            # tree level 1: j0 + j1 -> smt cols [p*1024, +1024)
            with nc.allow_low_precision(reason="fp16 neighbor sums"):
                nc.vector.tensor_add(smt[:, p * 1024:(p + 1) * 1024],
                                     pdt[:, base:base + 1024],
                                     pdt[:, base + 1024:base + 2048])
            return pra

        pend = {}  # chunk -> smt awaiting tree levels 2+3

        def emit_l23(cc):
            smt = pend.pop(cc)
            sct = s_pool.tile([128, 1024], F16, tag="s", name=f"s_{cc}")
            with nc.allow_low_precision(reason="fp16 neighbor sums"):
                nc.vector.tensor_add(smt[:, 0:1024], smt[:, 0:1024],
                                     smt[:, 1024:2048])
                nc.vector.tensor_add(smt[:, 2048:3072], smt[:, 2048:3072],
                                     smt[:, 3072:4096])
                nc.vector.tensor_add(sct[:, :], smt[:, 0:1024],
                                     smt[:, 2048:3072])
            state[cc]["s"] = sct

        for c in range(NCH):
            xbt = x_pool.tile([128, 2048], F16, tag="x", name=f"x_{c}")
            hsc = hs_pool.tile([128, 8192], F16, tag="hs", name=f"hs_{c}")
            hs8c = h8_pool.tile([128, K_FP8_PAIRS * 2048], FP8, tag="h8",
                                name=f"h8_{c}")
            if c == 0:
                nc.sync.dma_start(out=xbt[:, 0:1024], in_=xbL[c][:, 0:1024])
                nc.sync.dma_start(out=wp_t[:, 512:WP_SPLIT],
                                  in_=wpL[:, 512:WP_SPLIT])
                nc.sync.dma_start(out=xbt[:, 1024:2048],
                                  in_=xbL[c][:, 1024:2048])
                nc.sync.dma_start(out=wp8_t[:, :], in_=wp8L[:, :])
                nc.sync.dma_start(out=hs8c[:, 0:2048], in_=hs8L[c][:, 0:2048])
                nc.sync.dma_start(out=wp_t[:, WP_SPLIT:WP_COLS],
                                  in_=wpL[:, WP_SPLIT:WP_COLS])
                nc.sync.dma_start(out=hs8c[:, 2048:],
                                  in_=hs8L[c][:, 2048:])
                nc.sync.dma_start(out=hsc[:, :], in_=hsL[c])
            else:
                nc.sync.dma_start(out=xbt[:, :], in_=xbL[c])
                nc.sync.dma_start(out=hs8c[:, :], in_=hs8L[c])
                nc.sync.dma_start(out=hsc[:, :], in_=hsL[c])

            if c > 0:
                emit_l23(c - 1)

            pg = pp_pool.tile([128, 2048], F32, tag="pp", name=f"pg_{c}")
            for fi in range(2):
                o = pg[:, fi * 512:(fi + 1) * 512]
                nc.tensor.matmul(o, wcol("wir", 0, fi), xbt[:, 0:512],
                                 start=True, stop=False)
                nc.tensor.matmul(o, wcol("wir", 1, fi), xbt[:, 512:1024],
                                 start=False, stop=True)
            for fi in range(2):
                o = pg[:, 1024 + fi * 512:1024 + (fi + 1) * 512]
                nc.tensor.matmul(o, wcol("wiz", 0, fi), xbt[:, 0:512],
                                 start=True, stop=False)
                nc.tensor.matmul(o, wcol("wiz", 1, fi), xbt[:, 512:1024],
                                 start=False, stop=False)
                nc.tensor.matmul(o, wcol("whz", 0, fi), xbt[:, 1024:1536],
                                 start=False, stop=False)
                nc.tensor.matmul(o, wcol("whz", 1, fi), xbt[:, 1536:2048],
                                 start=False, stop=True)
            # xr drain on ACT; bias b_r per f-chunk
            xrt = xr_pool.tile([128, 1024], F16, tag="xr", name=f"xr_{c}")
            for fi in range(2):
                nc.scalar.add(xrt[:, fi * 512:(fi + 1) * 512],
                              pg[:, fi * 512:(fi + 1) * 512],
                              bias_t[:, fi * 3:fi * 3 + 1])
            zt = z_pool.tile([128, 1024], F16, tag="z", name=f"z_{c}")
            for fi in range(2):
                nc.scalar.activation(zt[:, fi * 512:(fi + 1) * 512],
                                     pg[:, 1024 + fi * 512:1024 + (fi + 1) * 512],
                                     SIG, bias=bias_t[:, fi * 3 + 1:fi * 3 + 2])

            rct = rc_pool.tile([128, 4 * 2048], F16, tag="rc", name=f"rc_{c}")
            pdt = pd_pool.tile([128, 4 * 2048], F16, tag="pd", name=f"pd_{c}")
            smt = sm_pool.tile([128, 4 * 1024], F16, tag="sm", name=f"sm_{c}")
            state[c] = {"x": xbt, "hf": xbt[:, 1024:2048], "z": zt}
            emit_pair(c, 0, hsc, hs8c, xrt, rct, pdt, smt)
            emit_pair(c, 1, hsc, hs8c, xrt, rct, pdt, smt)
            if c == NCH - 1:
                with nc.allow_low_precision(reason="fp16 neighbor sums"):
                    nc.vector.tensor_add(smt[:, 0:1024], smt[:, 0:1024],
                                         smt[:, 1024:2048])
            if c > 0:
                emit_mid(c)
            emit_pair(c, 2, hsc, hs8c, xrt, rct, pdt, smt)
            if c == NCH - 1:
                with nc.allow_low_precision(reason="fp16 neighbor sums"):
                    nc.vector.tensor_add(smt[:, 1024:2048], smt[:, 0:1024],
                                         smt[:, 2048:3072])
            if c == NCH - 1:
                pra3 = emit_pair(c, 3, hsc, hs8c, xrt, rct, pdt, smt,
                                 mm_only=True)
                emit_last_tail(c, pra3, hsc, xrt, rct, pdt, smt)
            else:
                emit_pair(c, 3, hsc, hs8c, xrt, rct, pdt, smt)
                pend[c] = smt

    nc.compile()
    return nc


def _prep_inputs(x, h_sum, hs, Wir, bir, Whr, bhr, Wiz, biz, Whz, bhz,
                 Win, bin_, Whn, bhn):
    """Shard + pre-chunk to per-core, per-chunk feature-major HBM layouts."""
    f32 = np.float32
    x = np.asarray(x, f32)
    h = np.asarray(h_sum, f32)
    hs = np.asarray(hs, f32)

    wpack = np.zeros((128, WP_COLS), f32)
    for w, W in (("wir", Wir), ("whr", Whr), ("wiz", Wiz), ("whz", Whz),
                 ("win", Win), ("whn", Whn)):
        WT = np.asarray(W, f32).T  # [in, out]
        for k in range(2):
            wpack[:, W_OFF[w] + k * 256:W_OFF[w] + (k + 1) * 256] = \
                WT[k * 128:(k + 1) * 128, :]
    wpack[:, ID_OFF:ID_OFF + 128] = np.eye(128, dtype=f32)
    wpack_f16 = np.ascontiguousarray(wpack.astype(F16_NP))

    WhrT = np.asarray(Whr, f32).T
    wp8 = np.zeros((128, WP8_COLS), f32)
    for f in range(2):
        for k in range(2):
            wp8[:, f * 256 + k * 128:f * 256 + (k + 1) * 128] = \
                WhrT[k * 128:(k + 1) * 128, f * 128:(f + 1) * 128]
    wp8_f8 = np.ascontiguousarray(wp8.astype(F8_NP))

    b_r = np.asarray(bir, f32) + np.asarray(bhr, f32)
    b_z = np.asarray(biz, f32) + np.asarray(bhz, f32)
    b_n = np.asarray(bin_, f32) + np.asarray(bhn, f32)
    biasp = np.empty((128, 6), f32)
    for f in range(2):
        biasp[:, f * 3 + 0] = b_r[f * 128:(f + 1) * 128]
        biasp[:, f * 3 + 1] = b_z[f * 128:(f + 1) * 128]
        biasp[:, f * 3 + 2] = b_n[f * 128:(f + 1) * 128]

    in_maps = []
    for c in range(M):
        sl = slice(c * BL, (c + 1) * BL)
        xc = x[sl].reshape(NCH, CW, 2, 128).transpose(0, 3, 2, 1)
        hc = h[sl].reshape(NCH, CW, 2, 128).transpose(0, 3, 2, 1)
        xb = np.concatenate([xc.astype(F16_NP).reshape(NCH, 128, 1024),
                             hc.astype(F16_NP).reshape(NCH, 128, 1024)], axis=2)
        # hs: [8, BL, 256] -> [pr, j, ch, b, k, p] -> [ch, p, pr, j, k, b]
        hsc = hs[:, sl, :].reshape(NPAIR, 2, NCH, CW, 2, 128)
        hs_t = hsc.transpose(2, 5, 0, 1, 4, 3)  # [ch, p, pr, j, k, b]
        m = {
            "xbL": np.ascontiguousarray(xb),
            "hsL": np.ascontiguousarray(
                hs_t.astype(F16_NP).reshape(NCH, 128, 8192)),
            "hs8L": np.ascontiguousarray(
                hs_t[:, :, :K_FP8_PAIRS].astype(F8_NP).reshape(
                    NCH, 128, K_FP8_PAIRS * 2048)),
            "wpL": wpack_f16,
            "wp8L": wp8_f8,
            "biasp": biasp,
        }
        in_maps.append(m)
    return in_maps


def _run(inputs, trace=False, **trace_kwargs):
    global _cached
    if _cached is None:
        _cached = _build()
    nc = _cached
    in_maps = _prep_inputs(**inputs)
    res = run_bass_kernel_spmd(nc, in_maps, list(range(M)), trace=trace,
                               **trace_kwargs)
    out = np.empty((B, H), np.float32)
    for c in range(M):
        # outL [ch, p, (f b)] -> [ch, b, f, p] -> [BL, 256]
        o = np.asarray(res.results[c]["outL"], np.float32).reshape(
            NCH, 128, 2, CW)
        out[c * BL:(c + 1) * BL, :] = o.transpose(0, 3, 2, 1).reshape(BL, 256)
    return out, res


def kernel(**inputs):
    return _run(inputs)[0]


# revision 36
# speedup vs baseline: 1.4813x; 1.0819x over previous
"""GRU-style GNN message-passing kernel for Trainium2 (8 NeuronCores, SPMD).

Reference computation (per node b, features 256, 8 neighbors):
    xr = x @ Wir.T + bir
    hr_n = hs_n @ Whr.T + bhr
    r_n = sigmoid(xr + hr_n)
    z = sigmoid(x @ Wiz.T + biz + h_sum @ Whz.T + bhz)
    s = sum_n r_n * hs_n
    n = tanh(x @ Win.T + bin + s @ Whn.T + bhn)
    out = (1 - z) * n + z * h_sum

Strategy: data-parallel over B=32768 across 8 cores (4096 rows each),
8 batch-chunks of 512 per core, feature-major on-chip layout
([256 features = 2 partition chunks of 128, batch free dim]).

Vs the bf16 baseline:
  - fp16 everywhere instead of bf16 (same engine throughput, 10-bit
    mantissa) which drops the base numeric error ~9x and frees error
    budget for:
  - fp8e4m3 DoubleRow matmuls for neighbor pairs 0..K_FP8_PAIRS-1 of
    the hr path: one 256-contraction DR matmul replaces two fp16
    passes (2x PE on those matmuls).  Whr is packed fp8 [f][k][m];
    hs additionally lands in SBUF as fp8 [pair,j,k,b] for those pairs.
  - hs on-chip layout [pair, j, k, b] (j-major) so the level-1 product
    tree add is one 1024-wide DVE op per pair.
  - no separate f32 h_sum copy: the combine reads the fp16 h_sum half
    of the packed x tile; output is stored fp16 and widened on host.
  - elementwise rebalance: combine sub+mul on Pool(gpsimd), final add
    on DVE.

The schedule keeps the chunk pipeline dense on PE (any PE idle gap
re-engages the HAM clock throttle): a 12-matmul front block (xr | z)
as the PE runway, neighbor pairs 0-1, the deferred n-gate of chunk
c-1, pairs 2-3; the product tree tail of chunk c runs at the front of
chunk c+1.
"""

import sys
import numpy as np
from contextlib import ExitStack

sys.path.insert(0, "/opt/trn_rl_repo")

import ml_dtypes
import concourse.bacc as bacc
import concourse.tile as tile
from concourse import mybir
from concourse.bass_utils import run_bass_kernel_spmd

F32 = mybir.dt.float32
F16 = mybir.dt.float16
FP8 = mybir.dt.float8e4
F16_NP = np.float16
F8_NP = ml_dtypes.float8_e4m3

N_NEIGH, B, IN, H = 8, 32768, 256, 256
M = 8                    # cores
BL = B // M              # rows per core (4096)
NCH = 8                  # batch chunks per core
CW = BL // NCH           # chunk width (512)
NPAIR = N_NEIGH // 2     # neighbor pairs (4)
K_FP8_PAIRS = 4          # pairs 0..K-1 use fp8 DoubleRow hr matmuls

_cached = None  # compiled program, reused across kernel() calls

SIG = mybir.ActivationFunctionType.Sigmoid
TANH = mybir.ActivationFunctionType.Tanh
DR = mybir.MatmulPerfMode.DoubleRow

# fp16 weight pack column offsets (need-ordered: xr gate, z gate,
# pair-3 whr, n).  Layout per weight: [k][f][m] (wcol below).
W_OFF = {"wir": 0, "wiz": 512, "whz": 1024, "whr": 1536, "win": 2176,
         "whn": 2688}
ID_OFF = 2048
WP_COLS = 3200
WP_SPLIT = 1536  # piece A: wir/wiz/whz; piece B: whr16/id/win/whn
# fp8 pack: whr for DR, [f][k][m]: col = f*256 + k*128 + m
WP8_COLS = 512


def _build():
    nc = bacc.Bacc("TRN2", target_bir_lowering=False, debug=False, num_devices=M)

    # xbL packs x (cols 0:1024) and h_sum (cols 1024:2048), fp16, k-major
    xbL = nc.dram_tensor("xbL", [NCH, 128, 2048], F16, kind="ExternalInput").ap()
    # hs fp16, per chunk [pair, j, k, b]
    hsL = nc.dram_tensor("hsL", [NCH, 128, 8192], F16, kind="ExternalInput").ap()
    # hs fp8 for DR pairs, per chunk [pair, j, k, b], pairs 0..K-1
    hs8L = nc.dram_tensor("hs8L", [NCH, 128, K_FP8_PAIRS * 2048], FP8,
                          kind="ExternalInput").ap()
    wpL = nc.dram_tensor("wpL", [128, WP_COLS], F16, kind="ExternalInput").ap()
    wp8L = nc.dram_tensor("wp8L", [128, WP8_COLS], FP8, kind="ExternalInput").ap()
    # bias pack: col f*3+j holds feature-chunk f of (b_r, b_z, b_n)[j]
    biasp = nc.dram_tensor("biasp", [128, 6], F32, kind="ExternalInput").ap()
    outL = nc.dram_tensor("outL", [NCH, 128, 1024], F16, kind="ExternalOutput").ap()

    with tile.TileContext(nc) as tc, ExitStack() as ctx:
        cpool = ctx.enter_context(tc.tile_pool(name="const", bufs=1))
        x_pool = ctx.enter_context(tc.tile_pool(name="x", bufs=3))
        hs_pool = ctx.enter_context(tc.tile_pool(name="hs", bufs=3))
        h8_pool = ctx.enter_context(tc.tile_pool(name="h8", bufs=3))
        xr_pool = ctx.enter_context(tc.tile_pool(name="xr", bufs=2))
        z_pool = ctx.enter_context(tc.tile_pool(name="z", bufs=2))
        rc_pool = ctx.enter_context(tc.tile_pool(name="rc", bufs=2))
        pd_pool = ctx.enter_context(tc.tile_pool(name="pd", bufs=2))
        sm_pool = ctx.enter_context(tc.tile_pool(name="sm", bufs=2))
        s_pool = ctx.enter_context(tc.tile_pool(name="s", bufs=2))
        n_pool = ctx.enter_context(tc.tile_pool(name="n", bufs=2))
        d_pool = ctx.enter_context(tc.tile_pool(name="d", bufs=2))
        o_pool = ctx.enter_context(tc.tile_pool(name="o", bufs=2))
        pp_pool = ctx.enter_context(tc.tile_pool(name="pp", bufs=2, space="PSUM"))

        # --- constants: weight packs + biases ---
        wp_t = cpool.tile([128, WP_COLS], F16, tag="wp", name="wp_t")
        nc.sync.dma_start(out=wp_t[:, 0:512], in_=wpL[:, 0:512])
        wp8_t = cpool.tile([128, WP8_COLS], FP8, tag="wp8", name="wp8_t")
        bias_t = cpool.tile([128, 6], F32, tag="biasp", name="bias_t")
        nc.sync.dma_start(out=bias_t[:, :], in_=biasp[:, :])

        # warm-up: the PE HAM clock-gate needs ~3.4us of sustained activity
        # to lift the 1.2GHz cold throttle.
        wu_t = cpool.tile([128, 128], F16, tag="wu", name="wu_t")
        nc.vector.memset(wu_t[:, :], 0)
        pwu = pp_pool.tile([128, 2048], F32, tag="pp", name="pwu")
        for i in range(44):
            nc.tensor.matmul(pwu[:, (i % 4) * 512:(i % 4) * 512 + 128],
                             wu_t[:, :], wu_t[:, :], start=True, stop=True)

        def wcol(w, k, f):  # stationary [128,128] for weight w, k-chunk, f-chunk
            off = W_OFF[w] + k * 256 + f * 128
            return wp_t[:, off:off + 128]

        def w8col(f):  # DR stationary [128, 2, 128] for whr fp8, f-chunk
            return wp8_t[:, f * 256:(f + 1) * 256].rearrange(
                "p (k m) -> p k m", k=2)

        id_t = wp_t[:, ID_OFF:ID_OFF + 128]

        state = {}  # chunk -> tiles needed by the deferred n-gate/combine

        def emit_ngate(c, fi, pn, st):
            o = pn[:, fi * 512:(fi + 1) * 512]
            nc.tensor.matmul(o, wcol("win", 0, fi), st["x"][:, 0:512],
                             start=True, stop=False)
            nc.tensor.matmul(o, wcol("win", 1, fi), st["x"][:, 512:1024],
                             start=False, stop=False)
            nc.tensor.matmul(o, wcol("whn", 0, fi), st["s"][:, 0:512],
                             start=False, stop=False)
            nc.tensor.matmul(o, wcol("whn", 1, fi), st["s"][:, 512:1024],
                             start=False, stop=True)

        def emit_mid(c):
            """Between pairs 1 and 2 of chunk c: the deferred n-gate of
            chunk c-1, then its combine (split Pool/DVE) + store."""
            st = state.pop(c - 1)
            pn = pp_pool.tile([128, 2048], F32, tag="pp", name=f"pn_{c - 1}")
            for fi in range(2):
                emit_ngate(c, fi, pn, st)
            nt = n_pool.tile([128, 1024], F16, tag="n", name=f"n_{c - 1}")
            for fi in range(2):
                nc.scalar.activation(nt[:, fi * 512:(fi + 1) * 512],
                                     pn[:, fi * 512:(fi + 1) * 512], TANH,
                                     bias=bias_t[:, fi * 3 + 2:fi * 3 + 3])
            # out = n + z * (h - n): sub+mul on Pool, final add on DVE
            dt_ = d_pool.tile([128, 1024], F16, tag="d", name=f"d_{c - 1}")
            with nc.allow_low_precision(reason="fp16 combine"):
                nc.gpsimd.tensor_sub(dt_[:, :], st["hf"][:, :], nt[:, :])
                nc.gpsimd.tensor_mul(dt_[:, :], st["z"][:, :], dt_[:, :])
                ot = o_pool.tile([128, 1024], F16, tag="o", name=f"o_{c - 1}")
                nc.vector.tensor_add(ot[:, :], nt[:, :], dt_[:, :])
            nc.sync.dma_start(out=outL[c - 1], in_=ot[:, :])

        def emit_last_tail(cc, pra3, hsc, xrt, rct, pdt, smt):
            """Final chunk tail, fully f-split: for each feature half, the
            pair-3 sigmoid/product/fold chain completes that half of s,
            feeding the matching Whn contraction chunk immediately.  The
            s-independent Win matmuls issue first so PE stays busy while
            ACT/DVE work through the f0 chain."""
            st = state.pop(cc)
            base = 3 * 2048
            pn = pp_pool.tile([128, 2048], F32, tag="pp", name=f"pn_{cc}")
            for fi in range(2):
                o = pn[:, fi * 512:(fi + 1) * 512]
                nc.tensor.matmul(o, wcol("win", 0, fi), st["x"][:, 0:512],
                                 start=True, stop=False)
                nc.tensor.matmul(o, wcol("win", 1, fi), st["x"][:, 512:1024],
                                 start=False, stop=False)
            sct = s_pool.tile([128, 1024], F16, tag="s", name=f"s_{cc}")
            nt = n_pool.tile([128, 1024], F16, tag="n", name=f"n_{cc}")
            dt_ = d_pool.tile([128, 1024], F16, tag="d", name=f"d_{cc}")
            ot = o_pool.tile([128, 1024], F16, tag="o", name=f"o_{cc}")
            for fi in range(2):
                fb = fi * 512
                # pair-3 sigmoid + products, this feature half of both j
                for j in range(2):
                    blk = slice(base + j * 1024 + fb, base + j * 1024 + fb + 512)
                    nc.scalar.activation(rct[:, blk],
                                         pra3[:, j * 1024 + fb:
                                              j * 1024 + fb + 512], SIG)
                    with nc.allow_low_precision(reason="fp16 products"):
                        nc.vector.tensor_mul(pdt[:, blk], rct[:, blk],
                                             hsc[:, blk])
                with nc.allow_low_precision(reason="fp16 neighbor sums"):
                    nc.vector.tensor_add(smt[:, 3072 + fb:3072 + fb + 512],
                                         pdt[:, base + fb:base + fb + 512],
                                         pdt[:, base + 1024 + fb:
                                             base + 1024 + fb + 512])
                    nc.vector.tensor_add(sct[:, fb:fb + 512],
                                         smt[:, 1024 + fb:1024 + fb + 512],
                                         smt[:, 3072 + fb:3072 + fb + 512])
                # Whn contraction chunk fi feeds both output halves
                for fo in range(2):
                    nc.tensor.matmul(pn[:, fo * 512:(fo + 1) * 512],
                                     wcol("whn", fi, fo), sct[:, fb:fb + 512],
                                     start=False, stop=(fi == 1))
            for fi in range(2):
                s_ = slice(fi * 512, (fi + 1) * 512)
                nc.scalar.activation(nt[:, s_], pn[:, s_], TANH,
                                     bias=bias_t[:, fi * 3 + 2:fi * 3 + 3])
                with nc.allow_low_precision(reason="fp16 combine"):
                    nc.vector.tensor_sub(dt_[:, s_], st["hf"][:, s_], nt[:, s_])
                    nc.vector.tensor_mul(dt_[:, s_], st["z"][:, s_], dt_[:, s_])
                    nc.vector.tensor_add(ot[:, s_], nt[:, s_], dt_[:, s_])
                nc.sync.dma_start(out=outL[cc][:, s_], in_=ot[:, s_])

        def emit_pair(c, p, hsc, hs8c, xrt, rct, pdt, smt, mm_only=False):
            """Neighbor pair p of chunk c.  PSUM pra layout [j, f, b]."""
            base = p * 2048
            pra = pp_pool.tile([128, 2048], F32, tag="pp", name=f"pr{p}_{c}")
            if p < K_FP8_PAIRS:
                for j in range(2):
                    mv = hs8c[:, base + j * 1024:base + (j + 1) * 1024]
                    mv3 = mv.rearrange("q (k b) -> q k b", k=2)
                    for fi in range(2):
                        nc.tensor.matmul(
                            pra[:, j * 1024 + fi * 512:j * 1024 + (fi + 1) * 512],
                            w8col(fi), mv3, start=True, stop=False,
                            perf_mode=DR)
            else:
                for j in range(2):
                    jb = base + j * 1024
                    for fi in range(2):
                        o = pra[:, j * 1024 + fi * 512:j * 1024 + (fi + 1) * 512]
                        nc.tensor.matmul(o, wcol("whr", 0, fi),
                                         hsc[:, jb:jb + 512],
                                         start=True, stop=False)
                        nc.tensor.matmul(o, wcol("whr", 1, fi),
                                         hsc[:, jb + 512:jb + 1024],
                                         start=False, stop=False)
            # xr identity adds (512-wide: matmul out must fit a PSUM bank)
            for j in range(2):
                for fi in range(2):
                    nc.tensor.matmul(
                        pra[:, j * 1024 + fi * 512:j * 1024 + (fi + 1) * 512],
                        id_t, xrt[:, fi * 512:(fi + 1) * 512],
                        start=False, stop=True)
            if mm_only:
                return pra
            # r for pair p, both neighbors in one activation
            nc.scalar.activation(rct[:, base:base + 2048], pra[:, :], SIG)
            blk = slice(base, base + 2048)
            with nc.allow_low_precision(reason="fp16 products"):
                nc.vector.tensor_mul(pdt[:, blk], rct[:, blk], hsc[:, blk])
            # tree level 1: j0 + j1 -> smt cols [p*1024, +1024)
            with nc.allow_low_precision(reason="fp16 neighbor sums"):
                nc.vector.tensor_add(smt[:, p * 1024:(p + 1) * 1024],
                                     pdt[:, base:base + 1024],
                                     pdt[:, base + 1024:base + 2048])
            return pra

        pend = {}  # chunk -> smt awaiting tree levels 2+3

        def emit_l23(cc):
            smt = pend.pop(cc)
            sct = s_pool.tile([128, 1024], F16, tag="s", name=f"s_{cc}")
            with nc.allow_low_precision(reason="fp16 neighbor sums"):
                nc.vector.tensor_add(smt[:, 0:1024], smt[:, 0:1024],
                                     smt[:, 1024:2048])
                nc.vector.tensor_add(smt[:, 2048:3072], smt[:, 2048:3072],
                                     smt[:, 3072:4096])
                nc.vector.tensor_add(sct[:, :], smt[:, 0:1024],
                                     smt[:, 2048:3072])
            state[cc]["s"] = sct

        for c in range(NCH):
            xbt = x_pool.tile([128, 2048], F16, tag="x", name=f"x_{c}")
            hsc = hs_pool.tile([128, 8192], F16, tag="hs", name=f"hs_{c}")
            hs8c = h8_pool.tile([128, K_FP8_PAIRS * 2048], FP8, tag="h8",
                                name=f"h8_{c}")
            if c == 0:
                nc.sync.dma_start(out=xbt[:, 0:1024], in_=xbL[c][:, 0:1024])
                nc.sync.dma_start(out=wp_t[:, 512:WP_SPLIT],
                                  in_=wpL[:, 512:WP_SPLIT])
                nc.sync.dma_start(out=xbt[:, 1024:2048],
                                  in_=xbL[c][:, 1024:2048])
                nc.sync.dma_start(out=wp8_t[:, :], in_=wp8L[:, :])
                nc.sync.dma_start(out=hs8c[:, 0:2048], in_=hs8L[c][:, 0:2048])
                nc.sync.dma_start(out=wp_t[:, WP_SPLIT:WP_COLS],
                                  in_=wpL[:, WP_SPLIT:WP_COLS])
                nc.sync.dma_start(out=hs8c[:, 2048:],
                                  in_=hs8L[c][:, 2048:])
                nc.sync.dma_start(out=hsc[:, :], in_=hsL[c])
            else:
                nc.sync.dma_start(out=xbt[:, :], in_=xbL[c])
                nc.sync.dma_start(out=hs8c[:, :], in_=hs8L[c])
                nc.sync.dma_start(out=hsc[:, :], in_=hsL[c])

            if c > 0:
                emit_l23(c - 1)

            pg = pp_pool.tile([128, 2048], F32, tag="pp", name=f"pg_{c}")
            for fi in range(2):
                o = pg[:, fi * 512:(fi + 1) * 512]
                nc.tensor.matmul(o, wcol("wir", 0, fi), xbt[:, 0:512],
                                 start=True, stop=False)
                nc.tensor.matmul(o, wcol("wir", 1, fi), xbt[:, 512:1024],
                                 start=False, stop=True)
            for fi in range(2):
                o = pg[:, 1024 + fi * 512:1024 + (fi + 1) * 512]
                nc.tensor.matmul(o, wcol("wiz", 0, fi), xbt[:, 0:512],
                                 start=True, stop=False)
                nc.tensor.matmul(o, wcol("wiz", 1, fi), xbt[:, 512:1024],
                                 start=False, stop=False)
                nc.tensor.matmul(o, wcol("whz", 0, fi), xbt[:, 1024:1536],
                                 start=False, stop=False)
                nc.tensor.matmul(o, wcol("whz", 1, fi), xbt[:, 1536:2048],
                                 start=False, stop=True)
            # xr drain on ACT; bias b_r per f-chunk
            xrt = xr_pool.tile([128, 1024], F16, tag="xr", name=f"xr_{c}")
            for fi in range(2):
                nc.scalar.add(xrt[:, fi * 512:(fi + 1) * 512],
                              pg[:, fi * 512:(fi + 1) * 512],
                              bias_t[:, fi * 3:fi * 3 + 1])
            zt = z_pool.tile([128, 1024], F16, tag="z", name=f"z_{c}")
            for fi in range(2):
                nc.scalar.activation(zt[:, fi * 512:(fi + 1) * 512],
                                     pg[:, 1024 + fi * 512:1024 + (fi + 1) * 512],
                                     SIG, bias=bias_t[:, fi * 3 + 1:fi * 3 + 2])

            rct = rc_pool.tile([128, 4 * 2048], F16, tag="rc", name=f"rc_{c}")
            pdt = pd_pool.tile([128, 4 * 2048], F16, tag="pd", name=f"pd_{c}")
            smt = sm_pool.tile([128, 4 * 1024], F16, tag="sm", name=f"sm_{c}")
            state[c] = {"x": xbt, "hf": xbt[:, 1024:2048], "z": zt}
            emit_pair(c, 0, hsc, hs8c, xrt, rct, pdt, smt)
            emit_pair(c, 1, hsc, hs8c, xrt, rct, pdt, smt)
            if c == NCH - 1:
                with nc.allow_low_precision(reason="fp16 neighbor sums"):
                    nc.vector.tensor_add(smt[:, 0:1024], smt[:, 0:1024],
                                         smt[:, 1024:2048])
            if c > 0:
                emit_mid(c)
            emit_pair(c, 2, hsc, hs8c, xrt, rct, pdt, smt)
            if c == NCH - 1:
                with nc.allow_low_precision(reason="fp16 neighbor sums"):
                    nc.vector.tensor_add(smt[:, 1024:2048], smt[:, 0:1024],
                                         smt[:, 2048:3072])
            if c == NCH - 1:
                pra3 = emit_pair(c, 3, hsc, hs8c, xrt, rct, pdt, smt,
                                 mm_only=True)
                emit_last_tail(c, pra3, hsc, xrt, rct, pdt, smt)
            else:
                emit_pair(c, 3, hsc, hs8c, xrt, rct, pdt, smt)
                pend[c] = smt

    nc.compile()
    return nc


def _prep_inputs(x, h_sum, hs, Wir, bir, Whr, bhr, Wiz, biz, Whz, bhz,
                 Win, bin_, Whn, bhn):
    """Shard + pre-chunk to per-core, per-chunk feature-major HBM layouts."""
    f32 = np.float32
    x = np.asarray(x, f32)
    h = np.asarray(h_sum, f32)
    hs = np.asarray(hs, f32)

    wpack = np.zeros((128, WP_COLS), f32)
    for w, W in (("wir", Wir), ("whr", Whr), ("wiz", Wiz), ("whz", Whz),
                 ("win", Win), ("whn", Whn)):
        WT = np.asarray(W, f32).T  # [in, out]
        for k in range(2):
            wpack[:, W_OFF[w] + k * 256:W_OFF[w] + (k + 1) * 256] = \
                WT[k * 128:(k + 1) * 128, :]
    wpack[:, ID_OFF:ID_OFF + 128] = np.eye(128, dtype=f32)
    wpack_f16 = np.ascontiguousarray(wpack.astype(F16_NP))

    WhrT = np.asarray(Whr, f32).T
    wp8 = np.zeros((128, WP8_COLS), f32)
    for f in range(2):
        for k in range(2):
            wp8[:, f * 256 + k * 128:f * 256 + (k + 1) * 128] = \
                WhrT[k * 128:(k + 1) * 128, f * 128:(f + 1) * 128]
    wp8_f8 = np.ascontiguousarray(wp8.astype(F8_NP))

    b_r = np.asarray(bir, f32) + np.asarray(bhr, f32)
    b_z = np.asarray(biz, f32) + np.asarray(bhz, f32)
    b_n = np.asarray(bin_, f32) + np.asarray(bhn, f32)
    biasp = np.empty((128, 6), f32)
    for f in range(2):
        biasp[:, f * 3 + 0] = b_r[f * 128:(f + 1) * 128]
        biasp[:, f * 3 + 1] = b_z[f * 128:(f + 1) * 128]
        biasp[:, f * 3 + 2] = b_n[f * 128:(f + 1) * 128]

    in_maps = []
    for c in range(M):
        sl = slice(c * BL, (c + 1) * BL)
        xc = x[sl].reshape(NCH, CW, 2, 128).transpose(0, 3, 2, 1)
        hc = h[sl].reshape(NCH, CW, 2, 128).transpose(0, 3, 2, 1)
        xb = np.concatenate([xc.astype(F16_NP).reshape(NCH, 128, 1024),
                             hc.astype(F16_NP).reshape(NCH, 128, 1024)], axis=2)
        # hs: [8, BL, 256] -> [pr, j, ch, b, k, p] -> [ch, p, pr, j, k, b]
        hsc = hs[:, sl, :].reshape(NPAIR, 2, NCH, CW, 2, 128)
        hs_t = hsc.transpose(2, 5, 0, 1, 4, 3)  # [ch, p, pr, j, k, b]
        m = {
            "xbL": np.ascontiguousarray(xb),
            "hsL": np.ascontiguousarray(
                hs_t.astype(F16_NP).reshape(NCH, 128, 8192)),
            "hs8L": np.ascontiguousarray(
                hs_t[:, :, :K_FP8_PAIRS].astype(F8_NP).reshape(
                    NCH, 128, K_FP8_PAIRS * 2048)),
            "wpL": wpack_f16,
            "wp8L": wp8_f8,
            "biasp": biasp,
        }
        in_maps.append(m)
    return in_maps


def _run(inputs, trace=False, **trace_kwargs):
    global _cached
    if _cached is None:
        _cached = _build()
    nc = _cached
    in_maps = _prep_inputs(**inputs)
    res = run_bass_kernel_spmd(nc, in_maps, list(range(M)), trace=trace,
                               **trace_kwargs)
    out = np.empty((B, H), np.float32)
    for c in range(M):
        # outL [ch, p, (f b)] -> [ch, b, f, p] -> [BL, 256]
        o = np.asarray(res.results[c]["outL"], np.float32).reshape(
            NCH, 128, 2, CW)
        out[c * BL:(c + 1) * BL, :] = o.transpose(0, 3, 2, 1).reshape(BL, 256)
    return out, res


def kernel(**inputs):
    return _run(inputs)[0]


# revision 37
# speedup vs baseline: 1.4946x; 1.0090x over previous
"""GRU-style GNN message-passing kernel for Trainium2 (8 NeuronCores, SPMD).

Reference computation (per node b, features 256, 8 neighbors):
    xr = x @ Wir.T + bir
    hr_n = hs_n @ Whr.T + bhr
    r_n = sigmoid(xr + hr_n)
    z = sigmoid(x @ Wiz.T + biz + h_sum @ Whz.T + bhz)
    s = sum_n r_n * hs_n
    n = tanh(x @ Win.T + bin + s @ Whn.T + bhn)
    out = (1 - z) * n + z * h_sum

Strategy: data-parallel over B=32768 across 8 cores (4096 rows each),
8 batch-chunks of 512 per core, feature-major on-chip layout
([256 features = 2 partition chunks of 128, batch free dim]).

Vs the bf16 baseline:
  - fp16 everywhere instead of bf16 (same engine throughput, 10-bit
    mantissa) which drops the base numeric error ~9x and frees error
    budget for:
  - fp8e4m3 DoubleRow matmuls for neighbor pairs 0..K_FP8_PAIRS-1 of
    the hr path: one 256-contraction DR matmul replaces two fp16
    passes (2x PE on those matmuls).  Whr is packed fp8 [f][k][m];
    hs additionally lands in SBUF as fp8 [pair,j,k,b] for those pairs.
  - hs on-chip layout [pair, j, k, b] (j-major) so the level-1 product
    tree add is one 1024-wide DVE op per pair.
  - no separate f32 h_sum copy: the combine reads the fp16 h_sum half
    of the packed x tile; output is stored fp16 and widened on host.
  - elementwise rebalance: combine sub+mul on Pool(gpsimd), final add
    on DVE.

The schedule keeps the chunk pipeline dense on PE (any PE idle gap
re-engages the HAM clock throttle): a 12-matmul front block (xr | z)
as the PE runway, neighbor pairs 0-1, the deferred n-gate of chunk
c-1, pairs 2-3; the product tree tail of chunk c runs at the front of
chunk c+1.
"""

import sys
import numpy as np
from contextlib import ExitStack

sys.path.insert(0, "/opt/trn_rl_repo")

import ml_dtypes
import concourse.bacc as bacc
import concourse.tile as tile
from concourse import mybir
from concourse.bass_utils import run_bass_kernel_spmd

F32 = mybir.dt.float32
F16 = mybir.dt.float16
FP8 = mybir.dt.float8e4
F16_NP = np.float16
F8_NP = ml_dtypes.float8_e4m3

N_NEIGH, B, IN, H = 8, 32768, 256, 256
M = 8                    # cores
BL = B // M              # rows per core (4096)
NCH = 8                  # batch chunks per core
CW = BL // NCH           # chunk width (512)
NPAIR = N_NEIGH // 2     # neighbor pairs (4)
K_FP8_PAIRS = 4          # pairs 0..K-1 use fp8 DoubleRow hr matmuls

_cached = None  # compiled program, reused across kernel() calls

SIG = mybir.ActivationFunctionType.Sigmoid
TANH = mybir.ActivationFunctionType.Tanh
DR = mybir.MatmulPerfMode.DoubleRow

# fp16 weight pack column offsets (need-ordered: xr gate, z gate,
# pair-3 whr, n).  Layout per weight: [k][f][m] (wcol below).
W_OFF = {"wir": 0, "wiz": 512, "whz": 1024, "whr": 1536, "win": 2176,
         "whn": 2688}
ID_OFF = 2048
WP_COLS = 3200
WP_SPLIT = 1536  # piece A: wir/wiz/whz; piece B: whr16/id/win/whn
# fp8 pack: whr for DR, [f][k][m]: col = f*256 + k*128 + m
WP8_COLS = 512


def _build():
    nc = bacc.Bacc("TRN2", target_bir_lowering=False, debug=False, num_devices=M)

    # xbL packs x (cols 0:1024) and h_sum (cols 1024:2048), fp16, k-major
    xbL = nc.dram_tensor("xbL", [NCH, 128, 2048], F16, kind="ExternalInput").ap()
    # hs fp16, per chunk [pair, j, k, b]
    hsL = nc.dram_tensor("hsL", [NCH, 128, 8192], F16, kind="ExternalInput").ap()
    # hs fp8 for DR pairs, per chunk [pair, j, k, b], pairs 0..K-1
    hs8L = nc.dram_tensor("hs8L", [NCH, 128, K_FP8_PAIRS * 2048], FP8,
                          kind="ExternalInput").ap()
    wpL = nc.dram_tensor("wpL", [128, WP_COLS], F16, kind="ExternalInput").ap()
    wp8L = nc.dram_tensor("wp8L", [128, WP8_COLS], FP8, kind="ExternalInput").ap()
    # bias pack: col f*3+j holds feature-chunk f of (b_r, b_z, b_n)[j]
    biasp = nc.dram_tensor("biasp", [128, 6], F32, kind="ExternalInput").ap()
    outL = nc.dram_tensor("outL", [NCH, 128, 1024], F16, kind="ExternalOutput").ap()

    with tile.TileContext(nc) as tc, ExitStack() as ctx:
        cpool = ctx.enter_context(tc.tile_pool(name="const", bufs=1))
        x_pool = ctx.enter_context(tc.tile_pool(name="x", bufs=3))
        hs_pool = ctx.enter_context(tc.tile_pool(name="hs", bufs=3))
        h8_pool = ctx.enter_context(tc.tile_pool(name="h8", bufs=3))
        xr_pool = ctx.enter_context(tc.tile_pool(name="xr", bufs=2))
        z_pool = ctx.enter_context(tc.tile_pool(name="z", bufs=2))
        rc_pool = ctx.enter_context(tc.tile_pool(name="rc", bufs=2))
        pd_pool = ctx.enter_context(tc.tile_pool(name="pd", bufs=2))
        sm_pool = ctx.enter_context(tc.tile_pool(name="sm", bufs=2))
        s_pool = ctx.enter_context(tc.tile_pool(name="s", bufs=2))
        n_pool = ctx.enter_context(tc.tile_pool(name="n", bufs=2))
        d_pool = ctx.enter_context(tc.tile_pool(name="d", bufs=2))
        o_pool = ctx.enter_context(tc.tile_pool(name="o", bufs=2))
        pp_pool = ctx.enter_context(tc.tile_pool(name="pp", bufs=2, space="PSUM"))

        # --- constants: weight packs + biases ---
        wp_t = cpool.tile([128, WP_COLS], F16, tag="wp", name="wp_t")
        nc.sync.dma_start(out=wp_t[:, 0:512], in_=wpL[:, 0:512])
        wp8_t = cpool.tile([128, WP8_COLS], FP8, tag="wp8", name="wp8_t")
        bias_t = cpool.tile([128, 6], F32, tag="biasp", name="bias_t")
        nc.sync.dma_start(out=bias_t[:, :], in_=biasp[:, :])

        # warm-up: the PE HAM clock-gate needs ~3.4us of sustained activity
        # to lift the 1.2GHz cold throttle.
        wu_t = cpool.tile([128, 128], F16, tag="wu", name="wu_t")
        nc.vector.memset(wu_t[:, :], 0)
        pwu = pp_pool.tile([128, 2048], F32, tag="pp", name="pwu")
        for i in range(44):
            nc.tensor.matmul(pwu[:, (i % 4) * 512:(i % 4) * 512 + 128],
                             wu_t[:, :], wu_t[:, :], start=True, stop=True)

        def wcol(w, k, f):  # stationary [128,128] for weight w, k-chunk, f-chunk
            off = W_OFF[w] + k * 256 + f * 128
            return wp_t[:, off:off + 128]

        def w8col(f):  # DR stationary [128, 2, 128] for whr fp8, f-chunk
            return wp8_t[:, f * 256:(f + 1) * 256].rearrange(
                "p (k m) -> p k m", k=2)

        id_t = wp_t[:, ID_OFF:ID_OFF + 128]

        state = {}  # chunk -> tiles needed by the deferred n-gate/combine

        def emit_ngate(c, fi, pn, st):
            o = pn[:, fi * 512:(fi + 1) * 512]
            nc.tensor.matmul(o, wcol("win", 0, fi), st["x"][:, 0:512],
                             start=True, stop=False)
            nc.tensor.matmul(o, wcol("win", 1, fi), st["x"][:, 512:1024],
                             start=False, stop=False)
            nc.tensor.matmul(o, wcol("whn", 0, fi), st["s"][:, 0:512],
                             start=False, stop=False)
            nc.tensor.matmul(o, wcol("whn", 1, fi), st["s"][:, 512:1024],
                             start=False, stop=True)

        def emit_mid(c):
            """Between pairs 1 and 2 of chunk c: the deferred n-gate of
            chunk c-1, then its combine (split Pool/DVE) + store."""
            st = state.pop(c - 1)
            pn = pp_pool.tile([128, 2048], F32, tag="pp", name=f"pn_{c - 1}")
            for fi in range(2):
                emit_ngate(c, fi, pn, st)
            nt = n_pool.tile([128, 1024], F16, tag="n", name=f"n_{c - 1}")
            for fi in range(2):
                nc.scalar.activation(nt[:, fi * 512:(fi + 1) * 512],
                                     pn[:, fi * 512:(fi + 1) * 512], TANH,
                                     bias=bias_t[:, fi * 3 + 2:fi * 3 + 3])
            # out = n + z * (h - n): sub+mul on Pool, final add on DVE
            dt_ = d_pool.tile([128, 1024], F16, tag="d", name=f"d_{c - 1}")
            with nc.allow_low_precision(reason="fp16 combine"):
                nc.gpsimd.tensor_sub(dt_[:, :], st["hf"][:, :], nt[:, :])
                nc.gpsimd.tensor_mul(dt_[:, :], st["z"][:, :], dt_[:, :])
                ot = o_pool.tile([128, 1024], F16, tag="o", name=f"o_{c - 1}")
                nc.vector.tensor_add(ot[:, :], nt[:, :], dt_[:, :])
            nc.sync.dma_start(out=outL[c - 1], in_=ot[:, :])

        def emit_last_tail(cc, pra3, hsc, xrt, rct, pdt, smt):
            """Final chunk tail, fully f-split: for each feature half, the
            pair-3 sigmoid/product/fold chain completes that half of s,
            feeding the matching Whn contraction chunk immediately.  The
            s-independent Win matmuls issue first so PE stays busy while
            ACT/DVE work through the f0 chain."""
            st = state.pop(cc)
            base = 3 * 2048
            pn = pp_pool.tile([128, 2048], F32, tag="pp", name=f"pn_{cc}")
            for fi in range(2):
                o = pn[:, fi * 512:(fi + 1) * 512]
                nc.tensor.matmul(o, wcol("win", 0, fi), st["x"][:, 0:512],
                                 start=True, stop=False)
                nc.tensor.matmul(o, wcol("win", 1, fi), st["x"][:, 512:1024],
                                 start=False, stop=False)
            sct = s_pool.tile([128, 1024], F16, tag="s", name=f"s_{cc}")
            nt = n_pool.tile([128, 1024], F16, tag="n", name=f"n_{cc}")
            dt_ = d_pool.tile([128, 1024], F16, tag="d", name=f"d_{cc}")
            ot = o_pool.tile([128, 1024], F16, tag="o", name=f"o_{cc}")
            for fi in range(2):
                fb = fi * 512
                # pair-3 sigmoid + products, this feature half of both j
                for j in range(2):
                    blk = slice(base + j * 1024 + fb, base + j * 1024 + fb + 512)
                    nc.scalar.activation(rct[:, blk],
                                         pra3[:, j * 1024 + fb:
                                              j * 1024 + fb + 512], SIG)
                    with nc.allow_low_precision(reason="fp16 products"):
                        nc.vector.tensor_mul(pdt[:, blk], rct[:, blk],
                                             hsc[:, blk])
                with nc.allow_low_precision(reason="fp16 neighbor sums"):
                    nc.vector.tensor_add(smt[:, 3072 + fb:3072 + fb + 512],
                                         pdt[:, base + fb:base + fb + 512],
                                         pdt[:, base + 1024 + fb:
                                             base + 1024 + fb + 512])
                    nc.vector.tensor_add(sct[:, fb:fb + 512],
                                         smt[:, 1024 + fb:1024 + fb + 512],
                                         smt[:, 3072 + fb:3072 + fb + 512])
                # Whn contraction chunk fi feeds both output halves
                for fo in range(2):
                    nc.tensor.matmul(pn[:, fo * 512:(fo + 1) * 512],
                                     wcol("whn", fi, fo), sct[:, fb:fb + 512],
                                     start=False, stop=(fi == 1))
            for fi in range(2):
                s_ = slice(fi * 512, (fi + 1) * 512)
                nc.scalar.activation(nt[:, s_], pn[:, s_], TANH,
                                     bias=bias_t[:, fi * 3 + 2:fi * 3 + 3])
                with nc.allow_low_precision(reason="fp16 combine"):
                    nc.vector.tensor_sub(dt_[:, s_], st["hf"][:, s_], nt[:, s_])
                    nc.vector.tensor_mul(dt_[:, s_], st["z"][:, s_], dt_[:, s_])
                    nc.vector.tensor_add(ot[:, s_], nt[:, s_], dt_[:, s_])
                nc.sync.dma_start(out=outL[cc][:, s_], in_=ot[:, s_])

        def emit_pair(c, p, hsc, hs8c, xrt, rct, pdt, smt, mm_only=False):
            """Neighbor pair p of chunk c.  PSUM pra layout [j, f, b]."""
            base = p * 2048
            pra = pp_pool.tile([128, 2048], F32, tag="pp", name=f"pr{p}_{c}")
            if p < K_FP8_PAIRS:
                for j in range(2):
                    mv = hs8c[:, base + j * 1024:base + (j + 1) * 1024]
                    mv3 = mv.rearrange("q (k b) -> q k b", k=2)
                    for fi in range(2):
                        nc.tensor.matmul(
                            pra[:, j * 1024 + fi * 512:j * 1024 + (fi + 1) * 512],
                            w8col(fi), mv3, start=True, stop=False,
                            perf_mode=DR)
            else:
                for j in range(2):
                    jb = base + j * 1024
                    for fi in range(2):
                        o = pra[:, j * 1024 + fi * 512:j * 1024 + (fi + 1) * 512]
                        nc.tensor.matmul(o, wcol("whr", 0, fi),
                                         hsc[:, jb:jb + 512],
                                         start=True, stop=False)
                        nc.tensor.matmul(o, wcol("whr", 1, fi),
                                         hsc[:, jb + 512:jb + 1024],
                                         start=False, stop=False)
            # xr identity adds (512-wide: matmul out must fit a PSUM bank)
            for j in range(2):
                for fi in range(2):
                    nc.tensor.matmul(
                        pra[:, j * 1024 + fi * 512:j * 1024 + (fi + 1) * 512],
                        id_t, xrt[:, fi * 512:(fi + 1) * 512],
                        start=False, stop=True)
            if mm_only:
                return pra
            # r for pair p, both neighbors in one activation
            nc.scalar.activation(rct[:, base:base + 2048], pra[:, :], SIG)
            blk = slice(base, base + 2048)
            with nc.allow_low_precision(reason="fp16 products"):
                nc.vector.tensor_mul(pdt[:, blk], rct[:, blk], hsc[:, blk])
            # tree level 1: j0 + j1 -> smt cols [p*1024, +1024)
            with nc.allow_low_precision(reason="fp16 neighbor sums"):
                nc.vector.tensor_add(smt[:, p * 1024:(p + 1) * 1024],
                                     pdt[:, base:base + 1024],
                                     pdt[:, base + 1024:base + 2048])
            return pra

        pend = {}  # chunk -> smt awaiting tree levels 2+3

        def emit_l23(cc):
            smt = pend.pop(cc)
            sct = s_pool.tile([128, 1024], F16, tag="s", name=f"s_{cc}")
            with nc.allow_low_precision(reason="fp16 neighbor sums"):
                nc.vector.tensor_add(smt[:, 0:1024], smt[:, 0:1024],
                                     smt[:, 1024:2048])
                nc.vector.tensor_add(smt[:, 2048:3072], smt[:, 2048:3072],
                                     smt[:, 3072:4096])
                nc.vector.tensor_add(sct[:, :], smt[:, 0:1024],
                                     smt[:, 2048:3072])
            state[cc]["s"] = sct

        for c in range(NCH):
            xbt = x_pool.tile([128, 2048], F16, tag="x", name=f"x_{c}")
            hsc = hs_pool.tile([128, 8192], F16, tag="hs", name=f"hs_{c}")
            hs8c = h8_pool.tile([128, K_FP8_PAIRS * 2048], FP8, tag="h8",
                                name=f"h8_{c}")
            if c == 0:
                nc.sync.dma_start(out=xbt[:, 0:1024], in_=xbL[c][:, 0:1024])
                nc.sync.dma_start(out=wp_t[:, 512:WP_SPLIT],
                                  in_=wpL[:, 512:WP_SPLIT])
                nc.sync.dma_start(out=xbt[:, 1024:2048],
                                  in_=xbL[c][:, 1024:2048])
                nc.sync.dma_start(out=wp8_t[:, :], in_=wp8L[:, :])
                nc.sync.dma_start(out=hs8c[:, 0:2048], in_=hs8L[c][:, 0:2048])
                nc.sync.dma_start(out=wp_t[:, WP_SPLIT:WP_COLS],
                                  in_=wpL[:, WP_SPLIT:WP_COLS])
                nc.sync.dma_start(out=hs8c[:, 2048:],
                                  in_=hs8L[c][:, 2048:])
                nc.sync.dma_start(out=hsc[:, :], in_=hsL[c])
            else:
                nc.sync.dma_start(out=xbt[:, :], in_=xbL[c])
                nc.sync.dma_start(out=hs8c[:, :], in_=hs8L[c])
                nc.sync.dma_start(out=hsc[:, :], in_=hsL[c])

            if c > 0:
                emit_l23(c - 1)

            pg = pp_pool.tile([128, 2048], F32, tag="pp", name=f"pg_{c}")
            for fi in range(2):
                o = pg[:, fi * 512:(fi + 1) * 512]
                nc.tensor.matmul(o, wcol("wir", 0, fi), xbt[:, 0:512],
                                 start=True, stop=False)
                nc.tensor.matmul(o, wcol("wir", 1, fi), xbt[:, 512:1024],
                                 start=False, stop=True)
            for fi in range(2):
                o = pg[:, 1024 + fi * 512:1024 + (fi + 1) * 512]
                nc.tensor.matmul(o, wcol("wiz", 0, fi), xbt[:, 0:512],
                                 start=True, stop=False)
                nc.tensor.matmul(o, wcol("wiz", 1, fi), xbt[:, 512:1024],
                                 start=False, stop=False)
                nc.tensor.matmul(o, wcol("whz", 0, fi), xbt[:, 1024:1536],
                                 start=False, stop=False)
                nc.tensor.matmul(o, wcol("whz", 1, fi), xbt[:, 1536:2048],
                                 start=False, stop=True)
            # xr drain on ACT; bias b_r per f-chunk
            xrt = xr_pool.tile([128, 1024], F16, tag="xr", name=f"xr_{c}")
            for fi in range(2):
                nc.scalar.add(xrt[:, fi * 512:(fi + 1) * 512],
                              pg[:, fi * 512:(fi + 1) * 512],
                              bias_t[:, fi * 3:fi * 3 + 1])
            zt = z_pool.tile([128, 1024], F16, tag="z", name=f"z_{c}")
            for fi in range(2):
                nc.scalar.activation(zt[:, fi * 512:(fi + 1) * 512],
                                     pg[:, 1024 + fi * 512:1024 + (fi + 1) * 512],
                                     SIG, bias=bias_t[:, fi * 3 + 1:fi * 3 + 2])

            rct = rc_pool.tile([128, 4 * 2048], F16, tag="rc", name=f"rc_{c}")
            pdt = pd_pool.tile([128, 4 * 2048], F16, tag="pd", name=f"pd_{c}")
            smt = sm_pool.tile([128, 4 * 1024], F16, tag="sm", name=f"sm_{c}")
            state[c] = {"x": xbt, "hf": xbt[:, 1024:2048], "z": zt}
            emit_pair(c, 0, hsc, hs8c, xrt, rct, pdt, smt)
            emit_pair(c, 1, hsc, hs8c, xrt, rct, pdt, smt)
            if c == NCH - 1:
                with nc.allow_low_precision(reason="fp16 neighbor sums"):
                    nc.vector.tensor_add(smt[:, 0:1024], smt[:, 0:1024],
                                         smt[:, 1024:2048])
            # pair 2 runs BEFORE the deferred n-gate: its sigmoid (whose
            # drain the next chunk's front PSUM tile waits on) then starts
            # ~1.8us earlier, and the front instead waits the cheap tanh
            emit_pair(c, 2, hsc, hs8c, xrt, rct, pdt, smt)
            if c == NCH - 1:
                with nc.allow_low_precision(reason="fp16 neighbor sums"):
                    nc.vector.tensor_add(smt[:, 1024:2048], smt[:, 0:1024],
                                         smt[:, 2048:3072])
            if c > 0:
                emit_mid(c)
            if c == NCH - 1:
                pra3 = emit_pair(c, 3, hsc, hs8c, xrt, rct, pdt, smt,
                                 mm_only=True)
                emit_last_tail(c, pra3, hsc, xrt, rct, pdt, smt)
            else:
                emit_pair(c, 3, hsc, hs8c, xrt, rct, pdt, smt)
                pend[c] = smt

    nc.compile()
    return nc


def _prep_inputs(x, h_sum, hs, Wir, bir, Whr, bhr, Wiz, biz, Whz, bhz,
                 Win, bin_, Whn, bhn):
    """Shard + pre-chunk to per-core, per-chunk feature-major HBM layouts."""
    f32 = np.float32
    x = np.asarray(x, f32)
    h = np.asarray(h_sum, f32)
    hs = np.asarray(hs, f32)

    wpack = np.zeros((128, WP_COLS), f32)
    for w, W in (("wir", Wir), ("whr", Whr), ("wiz", Wiz), ("whz", Whz),
                 ("win", Win), ("whn", Whn)):
        WT = np.asarray(W, f32).T  # [in, out]
        for k in range(2):
            wpack[:, W_OFF[w] + k * 256:W_OFF[w] + (k + 1) * 256] = \
                WT[k * 128:(k + 1) * 128, :]
    wpack[:, ID_OFF:ID_OFF + 128] = np.eye(128, dtype=f32)
    wpack_f16 = np.ascontiguousarray(wpack.astype(F16_NP))

    WhrT = np.asarray(Whr, f32).T
    wp8 = np.zeros((128, WP8_COLS), f32)
    for f in range(2):
        for k in range(2):
            wp8[:, f * 256 + k * 128:f * 256 + (k + 1) * 128] = \
                WhrT[k * 128:(k + 1) * 128, f * 128:(f + 1) * 128]
    wp8_f8 = np.ascontiguousarray(wp8.astype(F8_NP))

    b_r = np.asarray(bir, f32) + np.asarray(bhr, f32)
    b_z = np.asarray(biz, f32) + np.asarray(bhz, f32)
    b_n = np.asarray(bin_, f32) + np.asarray(bhn, f32)
    biasp = np.empty((128, 6), f32)
    for f in range(2):
        biasp[:, f * 3 + 0] = b_r[f * 128:(f + 1) * 128]
        biasp[:, f * 3 + 1] = b_z[f * 128:(f + 1) * 128]
        biasp[:, f * 3 + 2] = b_n[f * 128:(f + 1) * 128]

    in_maps = []
    for c in range(M):
        sl = slice(c * BL, (c + 1) * BL)
        xc = x[sl].reshape(NCH, CW, 2, 128).transpose(0, 3, 2, 1)
        hc = h[sl].reshape(NCH, CW, 2, 128).transpose(0, 3, 2, 1)
        xb = np.concatenate([xc.astype(F16_NP).reshape(NCH, 128, 1024),
                             hc.astype(F16_NP).reshape(NCH, 128, 1024)], axis=2)
        # hs: [8, BL, 256] -> [pr, j, ch, b, k, p] -> [ch, p, pr, j, k, b]
        hsc = hs[:, sl, :].reshape(NPAIR, 2, NCH, CW, 2, 128)
        hs_t = hsc.transpose(2, 5, 0, 1, 4, 3)  # [ch, p, pr, j, k, b]
        m = {
            "xbL": np.ascontiguousarray(xb),
            "hsL": np.ascontiguousarray(
                hs_t.astype(F16_NP).reshape(NCH, 128, 8192)),
            "hs8L": np.ascontiguousarray(
                hs_t[:, :, :K_FP8_PAIRS].astype(F8_NP).reshape(
                    NCH, 128, K_FP8_PAIRS * 2048)),
            "wpL": wpack_f16,
            "wp8L": wp8_f8,
            "biasp": biasp,
        }
        in_maps.append(m)
    return in_maps


def _run(inputs, trace=False, **trace_kwargs):
    global _cached
    if _cached is None:
        _cached = _build()
    nc = _cached
    in_maps = _prep_inputs(**inputs)
    res = run_bass_kernel_spmd(nc, in_maps, list(range(M)), trace=trace,
                               **trace_kwargs)
    out = np.empty((B, H), np.float32)
    for c in range(M):
        # outL [ch, p, (f b)] -> [ch, b, f, p] -> [BL, 256]
        o = np.asarray(res.results[c]["outL"], np.float32).reshape(
            NCH, 128, 2, CW)
        out[c * BL:(c + 1) * BL, :] = o.transpose(0, 3, 2, 1).reshape(BL, 256)
    return out, res


def kernel(**inputs):
    return _run(inputs)[0]


# revision 41
# speedup vs baseline: 1.5039x; 1.0063x over previous
"""GRU-style GNN message-passing kernel for Trainium2 (8 NeuronCores, SPMD).

Reference computation (per node b, features 256, 8 neighbors):
    xr = x @ Wir.T + bir
    hr_n = hs_n @ Whr.T + bhr
    r_n = sigmoid(xr + hr_n)
    z = sigmoid(x @ Wiz.T + biz + h_sum @ Whz.T + bhz)
    s = sum_n r_n * hs_n
    n = tanh(x @ Win.T + bin + s @ Whn.T + bhn)
    out = (1 - z) * n + z * h_sum

Strategy: data-parallel over B=32768 across 8 cores (4096 rows each),
8 batch-chunks of 512 per core, feature-major on-chip layout
([256 features = 2 partition chunks of 128, batch free dim]).

Vs the bf16 baseline:
  - fp16 everywhere instead of bf16 (same engine throughput, 10-bit
    mantissa) which drops the base numeric error ~9x and frees error
    budget for:
  - fp8e4m3 DoubleRow matmuls for neighbor pairs 0..K_FP8_PAIRS-1 of
    the hr path: one 256-contraction DR matmul replaces two fp16
    passes (2x PE on those matmuls).  Whr is packed fp8 [f][k][m];
    hs additionally lands in SBUF as fp8 [pair,j,k,b] for those pairs.
  - hs on-chip layout [pair, j, k, b] (j-major) so the level-1 product
    tree add is one 1024-wide DVE op per pair.
  - no separate f32 h_sum copy: the combine reads the fp16 h_sum half
    of the packed x tile; output is stored fp16 and widened on host.
  - elementwise rebalance: combine sub+mul on Pool(gpsimd), final add
    on DVE.

The schedule keeps the chunk pipeline dense on PE (any PE idle gap
re-engages the HAM clock throttle): a 12-matmul front block (xr | z)
as the PE runway, neighbor pairs 0-1, the deferred n-gate of chunk
c-1, pairs 2-3; the product tree tail of chunk c runs at the front of
chunk c+1.
"""

import sys
import numpy as np
from contextlib import ExitStack

sys.path.insert(0, "/opt/trn_rl_repo")

import ml_dtypes
import concourse.bacc as bacc
import concourse.tile as tile
from concourse import mybir
from concourse.bass_utils import run_bass_kernel_spmd

F32 = mybir.dt.float32
F16 = mybir.dt.float16
FP8 = mybir.dt.float8e4
F16_NP = np.float16
F8_NP = ml_dtypes.float8_e4m3

N_NEIGH, B, IN, H = 8, 32768, 256, 256
M = 8                    # cores
BL = B // M              # rows per core (4096)
NCH = 8                  # batch chunks per core
CW = BL // NCH           # chunk width (512)
NPAIR = N_NEIGH // 2     # neighbor pairs (4)
K_FP8_PAIRS = 4          # pairs 0..K-1 use fp8 DoubleRow hr matmuls

_cached = None  # compiled program, reused across kernel() calls

SIG = mybir.ActivationFunctionType.Sigmoid
TANH = mybir.ActivationFunctionType.Tanh
DR = mybir.MatmulPerfMode.DoubleRow

# fp16 weight pack column offsets (need-ordered: xr gate, z gate,
# pair-3 whr, n).  Layout per weight: [k][f][m] (wcol below).
W_OFF = {"wir": 0, "wiz": 512, "whz": 1024, "whr": 1536, "win": 2176,
         "whn": 2688}
ID_OFF = 2048
WP_COLS = 3200
WP_SPLIT = 1536  # piece A: wir/wiz/whz; piece B: whr16/id/win/whn
# fp8 pack: whr for DR, [f][k][m]: col = f*256 + k*128 + m
WP8_COLS = 512


def _build():
    nc = bacc.Bacc("TRN2", target_bir_lowering=False, debug=False, num_devices=M)

    # xbL packs x (cols 0:1024) and h_sum (cols 1024:2048), fp16, k-major
    xbL = nc.dram_tensor("xbL", [NCH, 128, 2048], F16, kind="ExternalInput").ap()
    # hs fp16, per chunk [pair, j, k, b]
    hsL = nc.dram_tensor("hsL", [NCH, 128, 8192], F16, kind="ExternalInput").ap()
    # hs fp8 for DR pairs, per chunk [pair, j, k, b], pairs 0..K-1
    hs8L = nc.dram_tensor("hs8L", [NCH, 128, K_FP8_PAIRS * 2048], FP8,
                          kind="ExternalInput").ap()
    wpL = nc.dram_tensor("wpL", [128, WP_COLS], F16, kind="ExternalInput").ap()
    wp8L = nc.dram_tensor("wp8L", [128, WP8_COLS], FP8, kind="ExternalInput").ap()
    # bias pack: col f*3+j holds feature-chunk f of (b_r, b_z, b_n)[j]
    biasp = nc.dram_tensor("biasp", [128, 6], F32, kind="ExternalInput").ap()
    outL = nc.dram_tensor("outL", [NCH, 128, 1024], F16, kind="ExternalOutput").ap()

    with tile.TileContext(nc) as tc, ExitStack() as ctx:
        cpool = ctx.enter_context(tc.tile_pool(name="const", bufs=1))
        x_pool = ctx.enter_context(tc.tile_pool(name="x", bufs=3))
        hs_pool = ctx.enter_context(tc.tile_pool(name="hs", bufs=3))
        h8_pool = ctx.enter_context(tc.tile_pool(name="h8", bufs=3))
        xr_pool = ctx.enter_context(tc.tile_pool(name="xr", bufs=2))
        z_pool = ctx.enter_context(tc.tile_pool(name="z", bufs=2))
        rc_pool = ctx.enter_context(tc.tile_pool(name="rc", bufs=2))
        pd_pool = ctx.enter_context(tc.tile_pool(name="pd", bufs=2))
        sm_pool = ctx.enter_context(tc.tile_pool(name="sm", bufs=2))
        s_pool = ctx.enter_context(tc.tile_pool(name="s", bufs=2))
        n_pool = ctx.enter_context(tc.tile_pool(name="n", bufs=2))
        d_pool = ctx.enter_context(tc.tile_pool(name="d", bufs=2))
        o_pool = ctx.enter_context(tc.tile_pool(name="o", bufs=2))
        pp_pool = ctx.enter_context(tc.tile_pool(name="pp", bufs=2, space="PSUM"))

        # --- constants: weight packs + biases ---
        wp_t = cpool.tile([128, WP_COLS], F16, tag="wp", name="wp_t")
        nc.sync.dma_start(out=wp_t[:, 0:512], in_=wpL[:, 0:512])
        wp8_t = cpool.tile([128, WP8_COLS], FP8, tag="wp8", name="wp8_t")
        bias_t = cpool.tile([128, 6], F32, tag="biasp", name="bias_t")
        nc.sync.dma_start(out=bias_t[:, :], in_=biasp[:, :])

        # warm-up: the PE HAM clock-gate needs ~3.4us of sustained activity
        # to lift the 1.2GHz cold throttle.
        wu_t = cpool.tile([128, 128], F16, tag="wu", name="wu_t")
        nc.vector.memset(wu_t[:, :], 0)
        pwu = pp_pool.tile([128, 2048], F32, tag="pp", name="pwu")
        for i in range(44):
            nc.tensor.matmul(pwu[:, (i % 4) * 512:(i % 4) * 512 + 128],
                             wu_t[:, :], wu_t[:, :], start=True, stop=True)

        def wcol(w, k, f):  # stationary [128,128] for weight w, k-chunk, f-chunk
            off = W_OFF[w] + k * 256 + f * 128
            return wp_t[:, off:off + 128]

        def w8col(f):  # DR stationary [128, 2, 128] for whr fp8, f-chunk
            return wp8_t[:, f * 256:(f + 1) * 256].rearrange(
                "p (k m) -> p k m", k=2)

        id_t = wp_t[:, ID_OFF:ID_OFF + 128]

        state = {}  # chunk -> tiles needed by the deferred n-gate/combine

        def emit_ngate(c, fi, pn, st):
            o = pn[:, fi * 512:(fi + 1) * 512]
            nc.tensor.matmul(o, wcol("win", 0, fi), st["x"][:, 0:512],
                             start=True, stop=False)
            nc.tensor.matmul(o, wcol("win", 1, fi), st["x"][:, 512:1024],
                             start=False, stop=False)
            nc.tensor.matmul(o, wcol("whn", 0, fi), st["s"][:, 0:512],
                             start=False, stop=False)
            nc.tensor.matmul(o, wcol("whn", 1, fi), st["s"][:, 512:1024],
                             start=False, stop=True)

        def emit_mid(c):
            """Between pairs 1 and 2 of chunk c: the deferred n-gate of
            chunk c-1, then its combine (split Pool/DVE) + store."""
            st = state.pop(c - 1)
            pn = pp_pool.tile([128, 2048], F32, tag="pp", name=f"pn_{c - 1}")
            for fi in range(2):
                emit_ngate(c, fi, pn, st)
            nt = n_pool.tile([128, 1024], F16, tag="n", name=f"n_{c - 1}")
            for fi in range(2):
                nc.scalar.activation(nt[:, fi * 512:(fi + 1) * 512],
                                     pn[:, fi * 512:(fi + 1) * 512], TANH,
                                     bias=bias_t[:, fi * 3 + 2:fi * 3 + 3])
            # out = n + z * (h - n): sub+mul on Pool, final add on DVE
            dt_ = d_pool.tile([128, 1024], F16, tag="d", name=f"d_{c - 1}")
            with nc.allow_low_precision(reason="fp16 combine"):
                nc.gpsimd.tensor_sub(dt_[:, :], st["hf"][:, :], nt[:, :])
                nc.gpsimd.tensor_mul(dt_[:, :], st["z"][:, :], dt_[:, :])
                ot = o_pool.tile([128, 1024], F16, tag="o", name=f"o_{c - 1}")
                nc.vector.tensor_add(ot[:, :], nt[:, :], dt_[:, :])
            nc.sync.dma_start(out=outL[c - 1], in_=ot[:, :])

        def emit_last_tail(cc, pra3, pg, zt, hsc, xrt, rct, pdt, smt):
            """Final chunk tail, fully f-split: for each feature half, the
            pair-3 sigmoid/product/fold chain completes that half of s,
            feeding the matching Whn contraction chunk immediately.  The
            s-independent Win matmuls issue first so PE stays busy while
            ACT/DVE work through the f0 chain."""
            st = state.pop(cc)
            base = 3 * 2048
            pn = pp_pool.tile([128, 2048], F32, tag="pp", name=f"pn_{cc}")
            for fi in range(2):
                o = pn[:, fi * 512:(fi + 1) * 512]
                nc.tensor.matmul(o, wcol("win", 0, fi), st["x"][:, 0:512],
                                 start=True, stop=False)
                nc.tensor.matmul(o, wcol("win", 1, fi), st["x"][:, 512:1024],
                                 start=False, stop=False)
            sct = s_pool.tile([128, 1024], F16, tag="s", name=f"s_{cc}")
            nt = n_pool.tile([128, 1024], F16, tag="n", name=f"n_{cc}")
            dt_ = d_pool.tile([128, 1024], F16, tag="d", name=f"d_{cc}")
            ot = o_pool.tile([128, 1024], F16, tag="o", name=f"o_{cc}")
            for fi in range(2):
                fb = fi * 512
                # pair-3 sigmoid + products, this feature half of both j
                for j in range(2):
                    blk = slice(base + j * 1024 + fb, base + j * 1024 + fb + 512)
                    nc.scalar.activation(rct[:, blk],
                                         pra3[:, j * 1024 + fb:
                                              j * 1024 + fb + 512], SIG)
                    with nc.allow_low_precision(reason="fp16 products"):
                        nc.vector.tensor_mul(pdt[:, blk], rct[:, blk],
                                             hsc[:, blk])
                with nc.allow_low_precision(reason="fp16 neighbor sums"):
                    nc.vector.tensor_add(smt[:, 3072 + fb:3072 + fb + 512],
                                         pdt[:, base + fb:base + fb + 512],
                                         pdt[:, base + 1024 + fb:
                                             base + 1024 + fb + 512])
                    nc.vector.tensor_add(sct[:, fb:fb + 512],
                                         smt[:, 1024 + fb:1024 + fb + 512],
                                         smt[:, 3072 + fb:3072 + fb + 512])
                # Whn contraction chunk fi feeds both output halves
                for fo in range(2):
                    nc.tensor.matmul(pn[:, fo * 512:(fo + 1) * 512],
                                     wcol("whn", fi, fo), sct[:, fb:fb + 512],
                                     start=False, stop=(fi == 1))
            for fi in range(2):
                s_ = slice(fi * 512, (fi + 1) * 512)
                nc.scalar.activation(nt[:, s_], pn[:, s_], TANH,
                                     bias=bias_t[:, fi * 3 + 2:fi * 3 + 3])
                # deferred z sigmoid for this half (overlaps the sub below)
                nc.scalar.activation(zt[:, s_], pg[:, 1024 + fi * 512:
                                                    1024 + (fi + 1) * 512],
                                     SIG, bias=bias_t[:, fi * 3 + 1:fi * 3 + 2])
                with nc.allow_low_precision(reason="fp16 combine"):
                    nc.vector.tensor_sub(dt_[:, s_], st["hf"][:, s_], nt[:, s_])
                    nc.vector.tensor_mul(dt_[:, s_], st["z"][:, s_], dt_[:, s_])
                    nc.vector.tensor_add(ot[:, s_], nt[:, s_], dt_[:, s_])
                nc.sync.dma_start(out=outL[cc][:, s_], in_=ot[:, s_])

        def emit_pair(c, p, hsc, hs8c, xrt, rct, pdt, smt, mm_only=False):
            """Neighbor pair p of chunk c.  PSUM pra layout [j, f, b]."""
            base = p * 2048
            pra = pp_pool.tile([128, 2048], F32, tag="pp", name=f"pr{p}_{c}")
            if p < K_FP8_PAIRS:
                for j in range(2):
                    mv = hs8c[:, base + j * 1024:base + (j + 1) * 1024]
                    mv3 = mv.rearrange("q (k b) -> q k b", k=2)
                    for fi in range(2):
                        nc.tensor.matmul(
                            pra[:, j * 1024 + fi * 512:j * 1024 + (fi + 1) * 512],
                            w8col(fi), mv3, start=True, stop=False,
                            perf_mode=DR)
            else:
                for j in range(2):
                    jb = base + j * 1024
                    for fi in range(2):
                        o = pra[:, j * 1024 + fi * 512:j * 1024 + (fi + 1) * 512]
                        nc.tensor.matmul(o, wcol("whr", 0, fi),
                                         hsc[:, jb:jb + 512],
                                         start=True, stop=False)
                        nc.tensor.matmul(o, wcol("whr", 1, fi),
                                         hsc[:, jb + 512:jb + 1024],
                                         start=False, stop=False)
            # xr identity adds (512-wide: matmul out must fit a PSUM bank)
            for j in range(2):
                for fi in range(2):
                    nc.tensor.matmul(
                        pra[:, j * 1024 + fi * 512:j * 1024 + (fi + 1) * 512],
                        id_t, xrt[:, fi * 512:(fi + 1) * 512],
                        start=False, stop=True)
            if mm_only:
                return pra
            # r for pair p, both neighbors in one activation
            nc.scalar.activation(rct[:, base:base + 2048], pra[:, :], SIG)
            blk = slice(base, base + 2048)
            with nc.allow_low_precision(reason="fp16 products"):
                nc.vector.tensor_mul(pdt[:, blk], rct[:, blk], hsc[:, blk])
            # tree level 1: j0 + j1 -> smt cols [p*1024, +1024)
            with nc.allow_low_precision(reason="fp16 neighbor sums"):
                nc.vector.tensor_add(smt[:, p * 1024:(p + 1) * 1024],
                                     pdt[:, base:base + 1024],
                                     pdt[:, base + 1024:base + 2048])
            return pra

        pend = {}  # chunk -> smt awaiting tree levels 2+3

        def emit_l23(cc):
            smt = pend.pop(cc)
            sct = s_pool.tile([128, 1024], F16, tag="s", name=f"s_{cc}")
            with nc.allow_low_precision(reason="fp16 neighbor sums"):
                nc.vector.tensor_add(smt[:, 0:1024], smt[:, 0:1024],
                                     smt[:, 1024:2048])
                nc.vector.tensor_add(smt[:, 2048:3072], smt[:, 2048:3072],
                                     smt[:, 3072:4096])
                nc.vector.tensor_add(sct[:, :], smt[:, 0:1024],
                                     smt[:, 2048:3072])
            state[cc]["s"] = sct

        for c in range(NCH):
            xbt = x_pool.tile([128, 2048], F16, tag="x", name=f"x_{c}")
            hsc = hs_pool.tile([128, 8192], F16, tag="hs", name=f"hs_{c}")
            hs8c = h8_pool.tile([128, K_FP8_PAIRS * 2048], FP8, tag="h8",
                                name=f"h8_{c}")
            if c == 0:
                nc.sync.dma_start(out=xbt[:, 0:1024], in_=xbL[c][:, 0:1024])
                nc.sync.dma_start(out=wp_t[:, 512:WP_SPLIT],
                                  in_=wpL[:, 512:WP_SPLIT])
                nc.sync.dma_start(out=xbt[:, 1024:2048],
                                  in_=xbL[c][:, 1024:2048])
                nc.sync.dma_start(out=wp8_t[:, :], in_=wp8L[:, :])
                nc.sync.dma_start(out=hs8c[:, 0:2048], in_=hs8L[c][:, 0:2048])
                nc.sync.dma_start(out=wp_t[:, WP_SPLIT:WP_COLS],
                                  in_=wpL[:, WP_SPLIT:WP_COLS])
                nc.sync.dma_start(out=hs8c[:, 2048:],
                                  in_=hs8L[c][:, 2048:])
                nc.sync.dma_start(out=hsc[:, :], in_=hsL[c])
            else:
                nc.sync.dma_start(out=xbt[:, :], in_=xbL[c])
                nc.sync.dma_start(out=hs8c[:, :], in_=hs8L[c])
                nc.sync.dma_start(out=hsc[:, :], in_=hsL[c])

            if c > 0:
                emit_l23(c - 1)

            pg = pp_pool.tile([128, 2048], F32, tag="pp", name=f"pg_{c}")
            for fi in range(2):
                o = pg[:, fi * 512:(fi + 1) * 512]
                nc.tensor.matmul(o, wcol("wir", 0, fi), xbt[:, 0:512],
                                 start=True, stop=False)
                nc.tensor.matmul(o, wcol("wir", 1, fi), xbt[:, 512:1024],
                                 start=False, stop=True)
            for fi in range(2):
                o = pg[:, 1024 + fi * 512:1024 + (fi + 1) * 512]
                nc.tensor.matmul(o, wcol("wiz", 0, fi), xbt[:, 0:512],
                                 start=True, stop=False)
                nc.tensor.matmul(o, wcol("wiz", 1, fi), xbt[:, 512:1024],
                                 start=False, stop=False)
                nc.tensor.matmul(o, wcol("whz", 0, fi), xbt[:, 1024:1536],
                                 start=False, stop=False)
                nc.tensor.matmul(o, wcol("whz", 1, fi), xbt[:, 1536:2048],
                                 start=False, stop=True)
            # xr drain on ACT; bias b_r per f-chunk
            xrt = xr_pool.tile([128, 1024], F16, tag="xr", name=f"xr_{c}")
            for fi in range(2):
                nc.scalar.add(xrt[:, fi * 512:(fi + 1) * 512],
                              pg[:, fi * 512:(fi + 1) * 512],
                              bias_t[:, fi * 3:fi * 3 + 1])
            zt = z_pool.tile([128, 1024], F16, tag="z", name=f"z_{c}")
            if c < NCH - 1:
                for fi in range(2):
                    nc.scalar.activation(
                        zt[:, fi * 512:(fi + 1) * 512],
                        pg[:, 1024 + fi * 512:1024 + (fi + 1) * 512],
                        SIG, bias=bias_t[:, fi * 3 + 1:fi * 3 + 2])
            # last chunk: z sigmoids deferred into the tail (they gate only
            # the combine) so the critical pair sigmoids clear ACT sooner;
            # pg has no successor chunk waiting on it

            rct = rc_pool.tile([128, 4 * 2048], F16, tag="rc", name=f"rc_{c}")
            pdt = pd_pool.tile([128, 4 * 2048], F16, tag="pd", name=f"pd_{c}")
            smt = sm_pool.tile([128, 4 * 1024], F16, tag="sm", name=f"sm_{c}")
            state[c] = {"x": xbt, "hf": xbt[:, 1024:2048], "z": zt}
            emit_pair(c, 0, hsc, hs8c, xrt, rct, pdt, smt)
            emit_pair(c, 1, hsc, hs8c, xrt, rct, pdt, smt)
            if c == NCH - 1:
                with nc.allow_low_precision(reason="fp16 neighbor sums"):
                    nc.vector.tensor_add(smt[:, 0:1024], smt[:, 0:1024],
                                         smt[:, 1024:2048])
            # pair 2 runs BEFORE the deferred n-gate: its sigmoid (whose
            # drain the next chunk's front PSUM tile waits on) then starts
            # ~1.8us earlier, and the front instead waits the cheap tanh
            emit_pair(c, 2, hsc, hs8c, xrt, rct, pdt, smt)
            if c == NCH - 1:
                with nc.allow_low_precision(reason="fp16 neighbor sums"):
                    nc.vector.tensor_add(smt[:, 1024:2048], smt[:, 0:1024],
                                         smt[:, 2048:3072])
            if c > 0:
                emit_mid(c)
            if c == NCH - 1:
                pra3 = emit_pair(c, 3, hsc, hs8c, xrt, rct, pdt, smt,
                                 mm_only=True)
                emit_last_tail(c, pra3, pg, zt, hsc, xrt, rct, pdt, smt)
            else:
                emit_pair(c, 3, hsc, hs8c, xrt, rct, pdt, smt)
                pend[c] = smt

    nc.compile()
    return nc


def _prep_inputs(x, h_sum, hs, Wir, bir, Whr, bhr, Wiz, biz, Whz, bhz,
                 Win, bin_, Whn, bhn):
    """Shard + pre-chunk to per-core, per-chunk feature-major HBM layouts."""
    f32 = np.float32
    x = np.asarray(x, f32)
    h = np.asarray(h_sum, f32)
    hs = np.asarray(hs, f32)

    wpack = np.zeros((128, WP_COLS), f32)
    for w, W in (("wir", Wir), ("whr", Whr), ("wiz", Wiz), ("whz", Whz),
                 ("win", Win), ("whn", Whn)):
        WT = np.asarray(W, f32).T  # [in, out]
        for k in range(2):
            wpack[:, W_OFF[w] + k * 256:W_OFF[w] + (k + 1) * 256] = \
                WT[k * 128:(k + 1) * 128, :]
    wpack[:, ID_OFF:ID_OFF + 128] = np.eye(128, dtype=f32)
    wpack_f16 = np.ascontiguousarray(wpack.astype(F16_NP))

    WhrT = np.asarray(Whr, f32).T
    wp8 = np.zeros((128, WP8_COLS), f32)
    for f in range(2):
        for k in range(2):
            wp8[:, f * 256 + k * 128:f * 256 + (k + 1) * 128] = \
                WhrT[k * 128:(k + 1) * 128, f * 128:(f + 1) * 128]
    wp8_f8 = np.ascontiguousarray(wp8.astype(F8_NP))

    b_r = np.asarray(bir, f32) + np.asarray(bhr, f32)
    b_z = np.asarray(biz, f32) + np.asarray(bhz, f32)
    b_n = np.asarray(bin_, f32) + np.asarray(bhn, f32)
    biasp = np.empty((128, 6), f32)
    for f in range(2):
        biasp[:, f * 3 + 0] = b_r[f * 128:(f + 1) * 128]
        biasp[:, f * 3 + 1] = b_z[f * 128:(f + 1) * 128]
        biasp[:, f * 3 + 2] = b_n[f * 128:(f + 1) * 128]

    in_maps = []
    for c in range(M):
        sl = slice(c * BL, (c + 1) * BL)
        xc = x[sl].reshape(NCH, CW, 2, 128).transpose(0, 3, 2, 1)
        hc = h[sl].reshape(NCH, CW, 2, 128).transpose(0, 3, 2, 1)
        xb = np.concatenate([xc.astype(F16_NP).reshape(NCH, 128, 1024),
                             hc.astype(F16_NP).reshape(NCH, 128, 1024)], axis=2)
        # hs: [8, BL, 256] -> [pr, j, ch, b, k, p] -> [ch, p, pr, j, k, b]
        hsc = hs[:, sl, :].reshape(NPAIR, 2, NCH, CW, 2, 128)
        hs_t = hsc.transpose(2, 5, 0, 1, 4, 3)  # [ch, p, pr, j, k, b]
        m = {
            "xbL": np.ascontiguousarray(xb),
            "hsL": np.ascontiguousarray(
                hs_t.astype(F16_NP).reshape(NCH, 128, 8192)),
            "hs8L": np.ascontiguousarray(
                hs_t[:, :, :K_FP8_PAIRS].astype(F8_NP).reshape(
                    NCH, 128, K_FP8_PAIRS * 2048)),
            "wpL": wpack_f16,
            "wp8L": wp8_f8,
            "biasp": biasp,
        }
        in_maps.append(m)
    return in_maps


def _run(inputs, trace=False, **trace_kwargs):
    global _cached
    if _cached is None:
        _cached = _build()
    nc = _cached
    in_maps = _prep_inputs(**inputs)
    res = run_bass_kernel_spmd(nc, in_maps, list(range(M)), trace=trace,
                               **trace_kwargs)
    out = np.empty((B, H), np.float32)
    for c in range(M):
        # outL [ch, p, (f b)] -> [ch, b, f, p] -> [BL, 256]
        o = np.asarray(res.results[c]["outL"], np.float32).reshape(
            NCH, 128, 2, CW)
        out[c * BL:(c + 1) * BL, :] = o.transpose(0, 3, 2, 1).reshape(BL, 256)
    return out, res


def kernel(**inputs):
    return _run(inputs)[0]
